# revision 4
# baseline (speedup 1.0000x reference)
"""Trainium2 Bass kernel for a pre-LN multi-head attention block (v2).

Full-input contract: kernel(**inputs) takes the unsharded tensors from
setup_inputs() and returns the full [4, 2048, 1024] output.

Sharding: 8 cores = 4 batches x 2 head-groups (8 heads each).
Each core computes LayerNorm(x[b]) (replicated within the batch pair),
its 8 heads of QKV + attention, and a partial projection
(attn_out_part @ w_proj_rows).  Host sums the two partials per batch and
adds b_proj + residual.

Host-side algebraic folds (exact):
  - ln_w folded into w_qkv columns, ln_b folded into b_qkv
  - softmax scale (0.125, exact in fp32/bf16) folded into W_q / b_q

v2 design (vs v1): keep the PE tensor engine gaplessly busy so it holds
its high p-state, and split softmax-exp across the Scalar (true Exp) and
Vector (Schraudolph bit-trick exp -> bf16 via int16 bias/scale) engines:
  LN:    one-pass ACT normalize h = Identity(x*rstd + (-mean*rstd)),
         PE-transpose batched 4-per-PSUM-tile, copies split ACT/DVE
  QKV:   V per token tile + QK per 512-token chunk, pipelined with LN;
         biases folded in as rank-1 ones-row matmuls (PSUM accumulated)
  Attn:  per (h,q-chunk) unit: 16 ST matmuls pairwise-interleaved with
         16 AV matmuls of the previous unit; exp of k-tile c on ACT for
         c < N_ACT_EXP else DVE Schraudolph; softmax sums via ones-row
         65th V column; normalization deferred two units (stage A: copy
         sums row + fast reciprocal + casts; stage B: PE broadcast
         matmul + DVE multiply) so the PE never waits on it
  Proj:  interleaved one [128,512] PSUM group per unit once a q-chunk's
         outputs are complete
"""

import sys

sys.path.insert(0, "/opt/trn_rl_repo")

import numpy as np
import ml_dtypes

import concourse.bass as bass
from concourse import bacc
import concourse.tile as tile
from concourse import mybir
from concourse.bass_utils import run_bass_kernel_spmd
from concourse.masks import make_identity

EMB = 1024
HEADS = 16
HD = 64
SCALE = HD ** -0.5
N_TOK = 2048
N_CORES = 8
HPC = 8                 # heads per core
QK_COLS = HPC * HD      # 512
P = 128
NT = N_TOK // P         # 16 token tiles
EC = EMB // P           # 8 emb chunks
QCH = 4                 # q chunks of 512
NKT = 16                # k tiles of 128
NPAIR = HPC // 2        # 4 head-pair tiles

BF16 = mybir.dt.bfloat16
F32 = mybir.dt.float32
I16 = mybir.dt.int16
AF = mybir.ActivationFunctionType
ALU = mybir.AluOpType

# Schraudolph exp -> bf16 bit pattern: e^s ~= bitcast_bf16(int16(
#   s * 2^7/ln2 + (127*2^7 - 366000/2^16))).  The uniform bias component
# cancels in softmax normalization; only the ~1.5% mantissa-interp ripple
# survives, which the residual-dominated output dilutes ~13x.
A_SCHR = 184.6650292
B_SCHR = 16250.4153
N_ACT_EXP = 8           # k-tiles of exp on ACT per unit; rest on DVE


def build_nc():
    nc = bacc.Bacc(trn_type="TRN2", target_bir_lowering=False)

    x_d = nc.dram_tensor("x", [N_TOK, EMB], F32, kind="ExternalInput")
    wq_d = nc.dram_tensor("wq", [EMB // 2, 2 * QK_COLS], BF16, kind="ExternalInput")
    wk_d = nc.dram_tensor("wk", [EMB // 2, 2 * QK_COLS], BF16, kind="ExternalInput")
    wv_d = nc.dram_tensor("wv", [EMB // 2, 2 * QK_COLS], BF16, kind="ExternalInput")
    bqr_d = nc.dram_tensor("bqr", [1, QK_COLS], BF16, kind="ExternalInput")
    bkr_d = nc.dram_tensor("bkr", [1, QK_COLS], BF16, kind="ExternalInput")
    bv_d = nc.dram_tensor("bv", [1, QK_COLS], BF16, kind="ExternalInput")
    wp_d = nc.dram_tensor("wp", [QK_COLS // 2, 2 * EMB], BF16, kind="ExternalInput")
    z_d = nc.dram_tensor("z", [N_TOK, EMB], F32, kind="ExternalOutput")

    with tile.TileContext(nc) as tc:
        _emit(nc, tc, x_d, wq_d, wk_d, wv_d, bqr_d, bkr_d, bv_d, wp_d, z_d)
    nc.finalize()
    return nc


def _emit(nc, tc, x_d, wq_d, wk_d, wv_d, bqr_d, bkr_d, bv_d, wp_d, z_d):
    from contextlib import ExitStack

    ctx = ExitStack()
    with ctx:
        consts = ctx.enter_context(tc.tile_pool(name="consts", bufs=1))
        persist = ctx.enter_context(tc.tile_pool(name="persist", bufs=1))

        ident = consts.tile([P, P], BF16, tag="ident", name="ident")
        make_identity(nc, ident)
        ones_row = consts.tile([1, 512], BF16, tag="ones_row", name="ones_row")
        nc.vector.memset(ones_row, 1.0)
        ones64 = consts.tile([1, HD], BF16, tag="ones64", name="ones64")
        nc.vector.memset(ones64, 1.0)
        eps_t = consts.tile([P, 1], F32, tag="eps", name="eps")
        nc.vector.memset(eps_t, 1e-5)

        bqr = consts.tile([1, QK_COLS], BF16, tag="bqr", name="bqr")
        nc.sync.dma_start(out=bqr, in_=bqr_d[:, :])
        bkr = consts.tile([1, QK_COLS], BF16, tag="bkr", name="bkr")
        nc.sync.dma_start(out=bkr, in_=bkr_d[:, :])
        bvt = consts.tile([1, QK_COLS], BF16, tag="bvt", name="bvt")
        nc.sync.dma_start(out=bvt, in_=bv_d[:, :])

        wq_s = []
        wk_s = []
        wv_s = []
        for c in range(EC // 2):
            for lst, srcd, nm in ((wq_s, wq_d, "wq"), (wk_s, wk_d, "wk"),
                                  (wv_s, wv_d, "wv")):
                t = persist.tile([P, 2, QK_COLS], BF16, tag=f"{nm}{c}", name=f"{nm}{c}")
                nc.sync.dma_start(out=t, in_=srcd[c * P:(c + 1) * P, :].rearrange(
                    "p (r m) -> p r m", r=2))
                lst.append(t)
        wp_s = []
        for i in range(2):
            t = persist.tile([P, 2, EMB], BF16, tag=f"wp{i}", name=f"wp{i}")
            nc.sync.dma_start(out=t, in_=wp_d[i * P:(i + 1) * P, :].rearrange(
                "p (r m) -> p r m", r=2))
            wp_s.append(t)

        qt = [persist.tile([P, N_TOK], BF16, tag=f"qt{i}", name=f"qt{i}") for i in range(NPAIR)]
        kt = [persist.tile([P, N_TOK], BF16, tag=f"kt{i}", name=f"kt{i}") for i in range(NPAIR)]
        otn = [persist.tile([P, 2, N_TOK], BF16, tag=f"otn{i}", name=f"otn{i}") for i in range(2)]
        vaug = [persist.tile([P, 2, HPC, HD + 1], BF16, tag=f"vaug{i}", name=f"vaug{i}")
                for i in range(NT // 2)]
        for t in range(NT // 2):
            nc.vector.memset(vaug[t][:, :, :, HD:HD + 1], 1.0)

        # ---------- Phase 1+2: LayerNorm + transpose + V + QK ----------
        ht_ctx = ExitStack()
        ht_pool = ht_ctx.enter_context(tc.tile_pool(name="ht", bufs=1))
        ht = [ht_pool.tile([P, 2, N_TOK], BF16, tag=f"ht{e}", name=f"ht{e}") for e in range(EC // 2)]

        with tc.tile_pool(name="ln", bufs=3) as ln_pool, \
             tc.tile_pool(name="hp", bufs=2) as hp, \
             tc.tile_pool(name="lns", bufs=4) as lns, \
             tc.tile_pool(name="ps_tr", bufs=2, space="PSUM") as ps_tr, \
             tc.tile_pool(name="ps_qkv", bufs=3, space="PSUM") as ps_qkv:
            for t in range(NT):
                x_t = ln_pool.tile([P, EMB], F32, tag="x", name="x")
                nc.sync.dma_start(out=x_t, in_=x_d[t * P:(t + 1) * P, :])
                stats = lns.tile([P, 2, 6], F32, tag="stats", name="stats")
                nc.vector.bn_stats(out=stats[:, 0, :], in_=x_t[:, 0:512])
                nc.vector.bn_stats(out=stats[:, 1, :], in_=x_t[:, 512:1024])
                mv = lns.tile([P, 2], F32, tag="mv", name="mv")
                nc.vector.bn_aggr(out=mv, in_=stats)
                sd = lns.tile([P, 1], F32, tag="sd", name="sd")
                nc.scalar.activation(out=sd, in_=mv[:, 1:2], func=AF.Sqrt,
                                     bias=eps_t, scale=1.0)
                rstd = lns.tile([P, 1], F32, tag="rstd", name="rstd")
                nc.vector.reciprocal(out=rstd, in_=sd)
                nmrs = lns.tile([P, 1], F32, tag="nmrs", name="nmrs")
                nc.vector.scalar_tensor_tensor(
                    out=nmrs, in0=mv[:, 0:1], scalar=-1.0, in1=rstd,
                    op0=ALU.mult, op1=ALU.mult)
                h_bf = hp.tile([P, EMB], BF16, tag="h", name="h")
                nc.scalar.activation(out=h_bf, in_=x_t, func=AF.Identity,
                                     bias=nmrs, scale=rstd)
                # transpose: 4 chunks per PSUM tile, copy out 2x2 chunks
                for half in range(2):
                    pt = ps_tr.tile([P, 4, P], BF16, tag="tr", name="tr")
                    for j in range(4):
                        e = 4 * half + j
                        nc.tensor.transpose(pt[:, j, :],
                                            h_bf[:, e * P:(e + 1) * P], ident)
                    dst0 = ht[2 * half][:, :, t * P:(t + 1) * P]
                    dst1 = ht[2 * half + 1][:, :, t * P:(t + 1) * P]
                    if half == 0:
                        nc.scalar.copy(out=dst0, in_=pt[:, 0:2, :])
                        nc.scalar.copy(out=dst1, in_=pt[:, 2:4, :])
                    else:
                        nc.vector.tensor_copy(out=dst0, in_=pt[:, 0:2, :])
                        nc.vector.tensor_copy(out=dst1, in_=pt[:, 2:4, :])
                # V for this token tile
                pv = ps_qkv.tile([P, 512], F32, tag="qkv", name="qkv")
                for c in range(EC // 2):
                    for r in range(2):
                        nc.tensor.matmul(pv, lhsT=ht[c][:, r, t * P:(t + 1) * P],
                                         rhs=wv_s[c][:, r, :],
                                         start=(c == 0 and r == 0), stop=False)
                nc.tensor.matmul(pv, lhsT=ones_row[:, 0:P], rhs=bvt,
                                 start=False, stop=True)
                nc.vector.tensor_copy(
                    out=vaug[t // 2][:, t % 2, :, 0:HD],
                    in_=pv.rearrange("p (h d) -> p h d", h=HPC))
                # QK for the completed 512-token chunk
                if t % 4 == 3:
                    n = t // 4
                    for m in range(NPAIR):
                        for w_s, brow, dst in ((wq_s, bqr, qt), (wk_s, bkr, kt)):
                            pq = ps_qkv.tile([P, 512], F32, tag="qkv", name="qkv")
                            for c in range(EC // 2):
                                for r in range(2):
                                    nc.tensor.matmul(
                                        pq, lhsT=w_s[c][:, r, m * P:(m + 1) * P],
                                        rhs=ht[c][:, r, n * 512:(n + 1) * 512],
                                        start=(c == 0 and r == 0), stop=False)
                            nc.tensor.matmul(pq, lhsT=brow[:, m * P:(m + 1) * P],
                                             rhs=ones_row, start=False, stop=True)
                            nc.scalar.copy(out=dst[m][:, n * 512:(n + 1) * 512],
                                           in_=pq)

        ht_ctx.close()

        # ---------------- Phase 3: attention (+ proj interleaved) -------
        with tc.tile_pool(name="expp", bufs=2) as expp, \
             tc.tile_pool(name="att_sm", bufs=3) as att_sm, \
             tc.tile_pool(name="zst", bufs=3) as zst, \
             tc.tile_pool(name="ps_st", bufs=4, space="PSUM") as ps_st, \
             tc.tile_pool(name="ps_ot", bufs=2, space="PSUM") as ps_ot, \
             tc.tile_pool(name="ps_misc", bufs=2, space="PSUM") as ps_misc:

            def emit_proj_half(q, ti, half):
                """One projection PSUM group: tokens [128], emb cols [512]."""
                tt = q * 4 + ti
                pz = ps_misc.tile([P, 512], F32, tag="misc", name="pz")
                for cc in range(2):
                    for rr in range(2):
                        nc.tensor.matmul(
                            pz, lhsT=otn[cc][:, rr, tt * P:(tt + 1) * P],
                            rhs=wp_s[cc][:, rr, half * 512:(half + 1) * 512],
                            start=(cc == 0 and rr == 0), stop=(cc == 1 and rr == 1))
                z_t = zst.tile([P, 512], F32, tag="z", name="z")
                nc.vector.tensor_copy(out=z_t, in_=pz)
                nc.sync.dma_start(
                    out=z_d[tt * P:(tt + 1) * P, half * 512:(half + 1) * 512],
                    in_=z_t)

            def stage_a(u):
                """After AV stop: extract sums row, 1/Z, casts, raw copy."""
                q, h, pot = u
                srow = att_sm.tile([1, 512], F32, tag="srow", name="srow")
                nc.scalar.copy(out=srow, in_=pot[HD:HD + 1, :])
                rec = att_sm.tile([1, 512], F32, tag="rec", name="rec")
                nc.vector.reciprocal_approx_fast(out=rec, in_=srow)
                rec_bf = att_sm.tile([1, 512], BF16, tag="rec_bf", name="rec_bf")
                nc.scalar.copy(out=rec_bf, in_=rec)
                ot_raw = att_sm.tile([HD, 512], BF16, tag="ot_raw", name="ot_raw")
                nc.scalar.copy(out=ot_raw, in_=pot[0:HD, :])
                return (q, h, rec_bf, ot_raw)

            def stage_b(u):
                """Two units later: broadcast 1/Z on PE, multiply into otn."""
                q, h, rec_bf, ot_raw = u
                pb = ps_misc.tile([HD, 512], F32, tag="misc", name="pb")
                nc.tensor.matmul(pb, lhsT=ones64, rhs=rec_bf,
                                 start=True, stop=True)
                nc.vector.tensor_mul(
                    otn[h // 4][(h % 2) * HD:(h % 2) * HD + HD, (h // 2) % 2,
                                q * 512:(q + 1) * 512],
                    ot_raw, pb)

            units = [(q, h) for q in range(QCH) for h in range(HPC)]
            av_prev = None      # (q, h, e_tiles) awaiting AV during this unit
            sa_queue = []       # stage-A results awaiting stage B (2-deep)
            proj_queue = []     # (q, ti, half) proj groups ready to emit

            def do_stage_b(sb):
                stage_b(sb)
                if sb[1] == HPC - 1:
                    proj_queue.extend(
                        (sb[0], ti, half) for ti in range(4) for half in range(2))

            for q, h in units:
                pair, prow = divmod(h, 2)
                prow *= HD
                # start-of-unit: one deferred normalize + one proj group.
                # both have inputs computed >= one full unit ago, so the PE
                # instructions here never wait on ACT/DVE.
                if len(sa_queue) >= 2:
                    do_stage_b(sa_queue.pop(0))
                if proj_queue:
                    emit_proj_half(*proj_queue.pop(0))
                e_tiles = [expp.tile([P, 512], BF16, tag=f"e{c}", name=f"e{c}")
                           for c in range(NKT)]
                pot_prev = None
                if av_prev is not None:
                    pot_prev = ps_ot.tile([HD + 1, 512], F32, tag="ot", name="ot")
                for c in range(NKT):
                    pst = ps_st.tile([P, 512], F32, tag="st", name="st")
                    nc.tensor.matmul(
                        pst,
                        lhsT=kt[pair][prow:prow + HD, c * P:(c + 1) * P],
                        rhs=qt[pair][prow:prow + HD, q * 512:(q + 1) * 512],
                        start=True, stop=True)
                    if av_prev is not None:
                        nc.tensor.matmul(
                            pot_prev, lhsT=vaug[c // 2][:, c % 2, av_prev[1], :],
                            rhs=av_prev[2][c], start=(c == 0), stop=(c == NKT - 1))
                    if c < N_ACT_EXP:
                        nc.scalar.activation(out=e_tiles[c], in_=pst, func=AF.Exp)
                    else:
                        nc.vector.tensor_scalar(
                            out=e_tiles[c].bitcast(I16), in0=pst,
                            scalar1=A_SCHR, scalar2=B_SCHR,
                            op0=ALU.mult, op1=ALU.add)
                # end-of-unit: AV of av_prev just completed -> stage A now
                if av_prev is not None:
                    sa_queue.append(stage_a((av_prev[0], av_prev[1], pot_prev)))
                av_prev = (q, h, e_tiles)

            # drain: AV for the last unit, then remaining finalize stages
            if len(sa_queue) >= 2:
                do_stage_b(sa_queue.pop(0))
            pot_last = ps_ot.tile([HD + 1, 512], F32, tag="ot", name="ot")
            for c in range(NKT):
                nc.tensor.matmul(
                    pot_last, lhsT=vaug[c // 2][:, c % 2, av_prev[1], :],
                    rhs=av_prev[2][c], start=(c == 0), stop=(c == NKT - 1))
            sa_queue.append(stage_a((av_prev[0], av_prev[1], pot_last)))
            while sa_queue:
                do_stage_b(sa_queue.pop(0))
                if proj_queue:
                    emit_proj_half(*proj_queue.pop(0))
            for g in proj_queue:
                emit_proj_half(*g)
            proj_queue.clear()


_CACHE = {}


def _get_nc():
    if "nc" not in _CACHE:
        _CACHE["nc"] = build_nc()
    return _CACHE["nc"]


def _prep_in_maps(x, ln_w, ln_b, w_qkv, b_qkv, w_proj, b_proj):
    bf = ml_dtypes.bfloat16
    x = np.asarray(x, np.float32)
    ln_w = np.asarray(ln_w, np.float32)
    ln_b = np.asarray(ln_b, np.float32)
    w_qkv = np.asarray(w_qkv, np.float32)
    b_qkv = np.asarray(b_qkv, np.float32)
    w_proj = np.asarray(w_proj, np.float32)

    b_eff = b_qkv + ln_b @ w_qkv
    w_eff = ln_w[:, None] * w_qkv
    w4 = w_eff.reshape(EMB, HEADS, HD, 3)
    b4 = b_eff.reshape(HEADS, HD, 3)
    wq = w4[..., 0] * SCALE
    wk = w4[..., 1]
    wv = w4[..., 2]
    bq = b4[..., 0] * SCALE
    bk = b4[..., 1]
    bv = b4[..., 2]

    def _dr(w):
        # [R, M] -> [R/2, 2M]: row 256c+128r+k -> (c*128+k, r*M+m)
        R, M = w.shape
        return np.ascontiguousarray(
            w.reshape(R // 256, 2, 128, M).transpose(0, 2, 1, 3).reshape(R // 2, 2 * M))

    in_maps = []
    for cid in range(N_CORES):
        bi, hg = divmod(cid, 2)
        hsl = slice(hg * HPC, (hg + 1) * HPC)
        in_maps.append({
            "x": np.ascontiguousarray(x[bi]),
            "wq": _dr(wq[:, hsl, :].reshape(EMB, QK_COLS)).astype(bf),
            "wk": _dr(wk[:, hsl, :].reshape(EMB, QK_COLS)).astype(bf),
            "wv": _dr(wv[:, hsl, :].reshape(EMB, QK_COLS)).astype(bf),
            "bqr": np.ascontiguousarray(
                bq[hsl].reshape(1, QK_COLS)).astype(bf),
            "bkr": np.ascontiguousarray(
                bk[hsl].reshape(1, QK_COLS)).astype(bf),
            "bv": np.ascontiguousarray(
                bv[hsl].reshape(1, QK_COLS)).astype(bf),
            "wp": _dr(w_proj[hg * QK_COLS:(hg + 1) * QK_COLS, :]).astype(bf),
        })
    return in_maps


def _gather(results, x, b_proj):
    b_proj = np.asarray(b_proj, np.float32)
    x = np.asarray(x, np.float32)
    out = np.empty((x.shape[0], N_TOK, EMB), np.float32)
    for bi in range(x.shape[0]):
        out[bi] = (results[2 * bi]["z"] + results[2 * bi + 1]["z"]
                   + b_proj[None, :] + x[bi])
    return out


def _run(inputs, **kw):
    in_maps = _prep_in_maps(**inputs)
    res = run_bass_kernel_spmd(_get_nc(), in_maps,
                               core_ids=list(range(N_CORES)), **kw)
    out = _gather(res.results, inputs["x"], inputs["b_proj"])
    return out, res


def kernel(**inputs):
    out, _ = _run(inputs)
    return out


# revision 10
# speedup vs baseline: 1.6825x; 1.6825x over previous
"""Trainium2 Bass kernel for a pre-LN multi-head attention block (v2).

Full-input contract: kernel(**inputs) takes the unsharded tensors from
setup_inputs() and returns the full [4, 2048, 1024] output.

Sharding: 8 cores = 4 batches x 2 head-groups (8 heads each).
Each core computes LayerNorm(x[b]) (replicated within the batch pair),
its 8 heads of QKV + attention, and a partial projection
(attn_out_part @ w_proj_rows).  Host sums the two partials per batch and
adds b_proj + residual.

Host-side algebraic folds (exact):
  - ln_w folded into w_qkv columns, ln_b folded into b_qkv
  - softmax scale (0.125, exact in fp32/bf16) folded into W_q / b_q

v2 design (vs v1): keep the PE tensor engine gaplessly busy so it holds
its high p-state, and split softmax-exp across the Scalar (true Exp) and
Vector (Schraudolph bit-trick exp -> bf16 via int16 bias/scale) engines:
  LN:    one-pass ACT normalize h = Identity(x*rstd + (-mean*rstd)),
         PE-transpose batched 4-per-PSUM-tile, copies split ACT/DVE
  QKV:   V per token tile + QK per 512-token chunk, pipelined with LN;
         biases folded in as rank-1 ones-row matmuls (PSUM accumulated)
  Attn:  per (h,q-chunk) unit: 16 ST matmuls pairwise-interleaved with
         16 AV matmuls of the previous unit; exp of k-tile c on ACT for
         c < N_ACT_EXP else DVE Schraudolph; softmax sums via ones-row
         65th V column; normalization deferred two units (stage A: copy
         sums row + fast reciprocal + casts; stage B: PE broadcast
         matmul + DVE multiply) so the PE never waits on it
  Proj:  interleaved one [128,512] PSUM group per unit once a q-chunk's
         outputs are complete
"""

import sys

sys.path.insert(0, "/opt/trn_rl_repo")

import numpy as np
import ml_dtypes

import concourse.bass as bass
from concourse import bacc
import concourse.tile as tile
from concourse import mybir
from concourse.bass_utils import run_bass_kernel_spmd
from concourse.masks import make_identity

EMB = 1024
HEADS = 16
HD = 64
SCALE = HD ** -0.5
N_TOK = 2048
N_CORES = 8
HPC = 8                 # heads per core
QK_COLS = HPC * HD      # 512
P = 128
NT = N_TOK // P         # 16 token tiles
EC = EMB // P           # 8 emb chunks
QCH = 4                 # q chunks of 512
NKT = 16                # k tiles of 128
NPAIR = HPC // 2        # 4 head-pair tiles

BF16 = mybir.dt.bfloat16
F32 = mybir.dt.float32
I16 = mybir.dt.int16
AF = mybir.ActivationFunctionType
ALU = mybir.AluOpType

# Schraudolph exp -> bf16 bit pattern: e^s ~= bitcast_bf16(int16(
#   s * 2^7/ln2 + (127*2^7 - 366000/2^16))).  The uniform bias component
# cancels in softmax normalization; only the ~1.5% mantissa-interp ripple
# survives, which the residual-dominated output dilutes ~13x.
A_SCHR = 184.6650292
B_SCHR = 16250.4153
N_ACT_EXP = 8           # k-tiles of exp on ACT per unit; rest on DVE


def build_nc():
    nc = bacc.Bacc(trn_type="TRN2", target_bir_lowering=False)

    x_d = nc.dram_tensor("x", [N_TOK, EMB], F32, kind="ExternalInput")
    wq_d = nc.dram_tensor("wq", [EMB // 2, 2 * QK_COLS], BF16, kind="ExternalInput")
    wk_d = nc.dram_tensor("wk", [EMB // 2, 2 * QK_COLS], BF16, kind="ExternalInput")
    wv_d = nc.dram_tensor("wv", [EMB // 2, 2 * QK_COLS], BF16, kind="ExternalInput")
    bqr_d = nc.dram_tensor("bqr", [1, QK_COLS], BF16, kind="ExternalInput")
    bkr_d = nc.dram_tensor("bkr", [1, QK_COLS], BF16, kind="ExternalInput")
    bv_d = nc.dram_tensor("bv", [1, QK_COLS], BF16, kind="ExternalInput")
    wp_d = nc.dram_tensor("wp", [QK_COLS // 2, 2 * EMB], BF16, kind="ExternalInput")
    z_d = nc.dram_tensor("z", [N_TOK, EMB], F32, kind="ExternalOutput")

    with tile.TileContext(nc) as tc:
        _emit(nc, tc, x_d, wq_d, wk_d, wv_d, bqr_d, bkr_d, bv_d, wp_d, z_d)
    nc.finalize()
    return nc


def _emit(nc, tc, x_d, wq_d, wk_d, wv_d, bqr_d, bkr_d, bv_d, wp_d, z_d):
    from contextlib import ExitStack

    ctx = ExitStack()
    with ctx:
        consts = ctx.enter_context(tc.tile_pool(name="consts", bufs=1))
        persist = ctx.enter_context(tc.tile_pool(name="persist", bufs=1))

        ident = consts.tile([P, P], BF16, tag="ident", name="ident")
        make_identity(nc, ident)
        ones_row = consts.tile([1, 512], BF16, tag="ones_row", name="ones_row")
        nc.vector.memset(ones_row, 1.0)
        ones_sq = consts.tile([P, P], BF16, tag="ones_sq", name="ones_sq")
        nc.vector.memset(ones_sq, 1.0)
        eps_t = consts.tile([P, 1], F32, tag="eps", name="eps")
        nc.vector.memset(eps_t, 1e-5)

        bqr = consts.tile([1, QK_COLS], BF16, tag="bqr", name="bqr")
        nc.sync.dma_start(out=bqr, in_=bqr_d[:, :])
        bkr = consts.tile([1, QK_COLS], BF16, tag="bkr", name="bkr")
        nc.sync.dma_start(out=bkr, in_=bkr_d[:, :])
        bvt = consts.tile([1, QK_COLS], BF16, tag="bvt", name="bvt")
        nc.sync.dma_start(out=bvt, in_=bv_d[:, :])

        wq_s = []
        wk_s = []
        wv_s = []
        for c in range(EC // 2):
            for lst, srcd, nm in ((wq_s, wq_d, "wq"), (wk_s, wk_d, "wk"),
                                  (wv_s, wv_d, "wv")):
                t = persist.tile([P, 2, QK_COLS], BF16, tag=f"{nm}{c}", name=f"{nm}{c}")
                nc.sync.dma_start(out=t, in_=srcd[c * P:(c + 1) * P, :].rearrange(
                    "p (r m) -> p r m", r=2))
                lst.append(t)
        wp_s = []
        for i in range(2):
            t = persist.tile([P, 2, EMB], BF16, tag=f"wp{i}", name=f"wp{i}")
            nc.sync.dma_start(out=t, in_=wp_d[i * P:(i + 1) * P, :].rearrange(
                "p (r m) -> p r m", r=2))
            wp_s.append(t)

        # qt: one zero-padded tile per head -- the other head's 64 rows stay
        # zero so ST matmuls can run with the full K=128 kt stationary and
        # keep a single (128,128,512) matmul shape throughout attention
        # (alternating stationary shapes serializes LDWEIGHTS, ~1.5x cost).
        qt = [persist.tile([P, N_TOK], BF16, tag=f"qt{i}", name=f"qt{i}") for i in range(HPC)]
        for i in range(HPC):
            nc.vector.memset(qt[i], 0.0)
        kt = [persist.tile([P, N_TOK], BF16, tag=f"kt{i}", name=f"kt{i}") for i in range(NPAIR)]
        otn = [persist.tile([P, 2, N_TOK], BF16, tag=f"otn{i}", name=f"otn{i}") for i in range(2)]
        # vaug M padded 65 -> 128 with zeros (col 64 = ones for softmax sums)
        vaug = [persist.tile([P, 2, HPC, P], BF16, tag=f"vaug{i}", name=f"vaug{i}")
                for i in range(NT // 2)]
        for t in range(NT // 2):
            nc.vector.memset(vaug[t][:, :, :, HD:], 0.0)
            nc.vector.memset(vaug[t][:, :, :, HD:HD + 1], 1.0)

        # ---------- Phase 1+2: LayerNorm + transpose + V + QK ----------
        ht_ctx = ExitStack()
        ht_pool = ht_ctx.enter_context(tc.tile_pool(name="ht", bufs=1))
        ht = [ht_pool.tile([P, 2, N_TOK], BF16, tag=f"ht{e}", name=f"ht{e}") for e in range(EC // 2)]

        with tc.tile_pool(name="ln", bufs=3) as ln_pool, \
             tc.tile_pool(name="hp", bufs=2) as hp, \
             tc.tile_pool(name="lns", bufs=4) as lns, \
             tc.tile_pool(name="ps_tr", bufs=2, space="PSUM") as ps_tr, \
             tc.tile_pool(name="ps_qkv", bufs=3, space="PSUM") as ps_qkv:
            for t in range(NT):
                x_t = ln_pool.tile([P, EMB], F32, tag="x", name="x")
                nc.sync.dma_start(out=x_t, in_=x_d[t * P:(t + 1) * P, :])
                stats = lns.tile([P, 2, 6], F32, tag="stats", name="stats")
                nc.vector.bn_stats(out=stats[:, 0, :], in_=x_t[:, 0:512])
                nc.vector.bn_stats(out=stats[:, 1, :], in_=x_t[:, 512:1024])
                mv = lns.tile([P, 2], F32, tag="mv", name="mv")
                nc.vector.bn_aggr(out=mv, in_=stats)
                sd = lns.tile([P, 1], F32, tag="sd", name="sd")
                nc.scalar.activation(out=sd, in_=mv[:, 1:2], func=AF.Sqrt,
                                     bias=eps_t, scale=1.0)
                rstd = lns.tile([P, 1], F32, tag="rstd", name="rstd")
                nc.vector.reciprocal(out=rstd, in_=sd)
                nmrs = lns.tile([P, 1], F32, tag="nmrs", name="nmrs")
                nc.vector.scalar_tensor_tensor(
                    out=nmrs, in0=mv[:, 0:1], scalar=-1.0, in1=rstd,
                    op0=ALU.mult, op1=ALU.mult)
                h_bf = hp.tile([P, EMB], BF16, tag="h", name="h")
                nc.scalar.activation(out=h_bf, in_=x_t, func=AF.Identity,
                                     bias=nmrs, scale=rstd)
                # transpose: 4 chunks per PSUM tile, copy out 2x2 chunks
                for half in range(2):
                    pt = ps_tr.tile([P, 4, P], BF16, tag="tr", name="tr")
                    for j in range(4):
                        e = 4 * half + j
                        nc.tensor.transpose(pt[:, j, :],
                                            h_bf[:, e * P:(e + 1) * P], ident)
                    dst0 = ht[2 * half][:, :, t * P:(t + 1) * P]
                    dst1 = ht[2 * half + 1][:, :, t * P:(t + 1) * P]
                    if half == 0:
                        nc.scalar.copy(out=dst0, in_=pt[:, 0:2, :])
                        nc.scalar.copy(out=dst1, in_=pt[:, 2:4, :])
                    else:
                        nc.vector.tensor_copy(out=dst0, in_=pt[:, 0:2, :])
                        nc.vector.tensor_copy(out=dst1, in_=pt[:, 2:4, :])
                # V for this token tile
                pv = ps_qkv.tile([P, 512], F32, tag="qkv", name="qkv")
                for c in range(EC // 2):
                    for r in range(2):
                        nc.tensor.matmul(pv, lhsT=ht[c][:, r, t * P:(t + 1) * P],
                                         rhs=wv_s[c][:, r, :],
                                         start=(c == 0 and r == 0), stop=False)
                nc.tensor.matmul(pv, lhsT=ones_row[:, 0:P], rhs=bvt,
                                 start=False, stop=True)
                nc.vector.tensor_copy(
                    out=vaug[t // 2][:, t % 2, :, 0:HD],
                    in_=pv.rearrange("p (h d) -> p h d", h=HPC))
                # QK for the completed 512-token chunk
                if t % 4 == 3:
                    n = t // 4
                    for m in range(NPAIR):
                        for w_s, brow, is_q in ((wq_s, bqr, True),
                                                (wk_s, bkr, False)):
                            pq = ps_qkv.tile([P, 512], F32, tag="qkv", name="qkv")
                            for c in range(EC // 2):
                                for r in range(2):
                                    nc.tensor.matmul(
                                        pq, lhsT=w_s[c][:, r, m * P:(m + 1) * P],
                                        rhs=ht[c][:, r, n * 512:(n + 1) * 512],
                                        start=(c == 0 and r == 0), stop=False)
                            nc.tensor.matmul(pq, lhsT=brow[:, m * P:(m + 1) * P],
                                             rhs=ones_row, start=False, stop=True)
                            sl = slice(n * 512, (n + 1) * 512)
                            if is_q:
                                nc.scalar.copy(out=qt[2 * m][0:HD, sl],
                                               in_=pq[0:HD, :])
                                nc.scalar.copy(out=qt[2 * m + 1][HD:P, sl],
                                               in_=pq[HD:P, :])
                            else:
                                nc.scalar.copy(out=kt[m][:, sl], in_=pq)

        ht_ctx.close()

        # ---------------- Phase 3: attention (+ proj interleaved) -------
        with tc.tile_pool(name="expp", bufs=2) as expp, \
             tc.tile_pool(name="att_sm", bufs=3) as att_sm, \
             tc.tile_pool(name="zst", bufs=3) as zst, \
             tc.tile_pool(name="ps_st", bufs=4, space="PSUM") as ps_st, \
             tc.tile_pool(name="ps_ot", bufs=2, space="PSUM") as ps_ot, \
             tc.tile_pool(name="ps_misc", bufs=2, space="PSUM") as ps_misc:

            def emit_proj_half(q, ti, half):
                """One projection PSUM group: tokens [128], emb cols [512]."""
                tt = q * 4 + ti
                pz = ps_misc.tile([P, 512], F32, tag="misc", name="pz")
                for cc in range(2):
                    for rr in range(2):
                        nc.tensor.matmul(
                            pz, lhsT=otn[cc][:, rr, tt * P:(tt + 1) * P],
                            rhs=wp_s[cc][:, rr, half * 512:(half + 1) * 512],
                            start=(cc == 0 and rr == 0), stop=(cc == 1 and rr == 1))
                z_t = zst.tile([P, 512], F32, tag="z", name="z")
                nc.vector.tensor_copy(out=z_t, in_=pz)
                nc.sync.dma_start(
                    out=z_d[tt * P:(tt + 1) * P, half * 512:(half + 1) * 512],
                    in_=z_t)

            # rec_pad slots: rows 1..127 must stay zero so the ones_sq
            # broadcast matmul (K=128, same shape as ST/AV) sees only row 0.
            for _ in range(3):
                rp = att_sm.tile([P, 512], BF16, tag="rec_pad", name="rec_pad")
                nc.vector.memset(rp, 0.0)

            def stage_a(u):
                """After AV stop: extract sums row, 1/Z, casts, raw copy."""
                q, h, pot = u
                srow = att_sm.tile([1, 512], F32, tag="srow", name="srow")
                nc.scalar.copy(out=srow, in_=pot[HD:HD + 1, :])
                rec = att_sm.tile([1, 512], F32, tag="rec", name="rec")
                nc.vector.reciprocal_approx_fast(out=rec, in_=srow)
                rec_pad = att_sm.tile([P, 512], BF16, tag="rec_pad", name="rec_pad")
                nc.scalar.copy(out=rec_pad[0:1, :], in_=rec)
                ot_raw = att_sm.tile([HD, 512], BF16, tag="ot_raw", name="ot_raw")
                nc.scalar.copy(out=ot_raw, in_=pot[0:HD, :])
                return (q, h, rec_pad, ot_raw)

            def stage_b(u):
                """Two units later: broadcast 1/Z on PE, multiply into otn."""
                q, h, rec_pad, ot_raw = u
                pb = ps_misc.tile([P, 512], F32, tag="misc", name="pb")
                nc.tensor.matmul(pb, lhsT=ones_sq, rhs=rec_pad,
                                 start=True, stop=True)
                nc.vector.tensor_mul(
                    otn[h // 4][(h % 2) * HD:(h % 2) * HD + HD, (h // 2) % 2,
                                q * 512:(q + 1) * 512],
                    ot_raw, pb[0:HD, :])

            units = [(q, h) for q in range(QCH) for h in range(HPC)]
            av_prev = None      # (q, h, e_tiles) awaiting AV during this unit
            sa_queue = []       # stage-A results awaiting stage B (2-deep)
            proj_queue = []     # (q, ti, half) proj groups ready to emit

            def do_stage_b(sb):
                stage_b(sb)
                if sb[1] == HPC - 1:
                    proj_queue.extend(
                        (sb[0], ti, half) for ti in range(4) for half in range(2))

            for q, h in units:
                pair = h // 2
                # start-of-unit: one deferred normalize + one proj group.
                # both have inputs computed >= one full unit ago, so the PE
                # instructions here never wait on ACT/DVE.
                if len(sa_queue) >= 2:
                    do_stage_b(sa_queue.pop(0))
                if proj_queue:
                    emit_proj_half(*proj_queue.pop(0))
                e_tiles = [expp.tile([P, 512], BF16, tag=f"e{c}", name=f"e{c}")
                           for c in range(NKT)]
                pot_prev = None
                if av_prev is not None:
                    pot_prev = ps_ot.tile([P, 512], F32, tag="ot", name="ot")
                for c in range(NKT):
                    pst = ps_st.tile([P, 512], F32, tag="st", name="st")
                    nc.tensor.matmul(
                        pst,
                        lhsT=kt[pair][:, c * P:(c + 1) * P],
                        rhs=qt[h][:, q * 512:(q + 1) * 512],
                        start=True, stop=True)
                    if av_prev is not None:
                        nc.tensor.matmul(
                            pot_prev, lhsT=vaug[c // 2][:, c % 2, av_prev[1], :],
                            rhs=av_prev[2][c], start=(c == 0), stop=(c == NKT - 1))
                    if c < N_ACT_EXP:
                        nc.scalar.activation(out=e_tiles[c], in_=pst, func=AF.Exp)
                    else:
                        nc.vector.tensor_scalar(
                            out=e_tiles[c].bitcast(I16), in0=pst,
                            scalar1=A_SCHR, scalar2=B_SCHR,
                            op0=ALU.mult, op1=ALU.add)
                # end-of-unit: AV of av_prev just completed -> stage A now
                if av_prev is not None:
                    sa_queue.append(stage_a((av_prev[0], av_prev[1], pot_prev)))
                av_prev = (q, h, e_tiles)

            # drain: AV for the last unit, then remaining finalize stages
            if len(sa_queue) >= 2:
                do_stage_b(sa_queue.pop(0))
            pot_last = ps_ot.tile([P, 512], F32, tag="ot", name="ot")
            for c in range(NKT):
                nc.tensor.matmul(
                    pot_last, lhsT=vaug[c // 2][:, c % 2, av_prev[1], :],
                    rhs=av_prev[2][c], start=(c == 0), stop=(c == NKT - 1))
            sa_queue.append(stage_a((av_prev[0], av_prev[1], pot_last)))
            while sa_queue:
                do_stage_b(sa_queue.pop(0))
                if proj_queue:
                    emit_proj_half(*proj_queue.pop(0))
            for g in proj_queue:
                emit_proj_half(*g)
            proj_queue.clear()


_CACHE = {}


def _get_nc():
    if "nc" not in _CACHE:
        _CACHE["nc"] = build_nc()
    return _CACHE["nc"]


def _prep_in_maps(x, ln_w, ln_b, w_qkv, b_qkv, w_proj, b_proj):
    bf = ml_dtypes.bfloat16
    x = np.asarray(x, np.float32)
    ln_w = np.asarray(ln_w, np.float32)
    ln_b = np.asarray(ln_b, np.float32)
    w_qkv = np.asarray(w_qkv, np.float32)
    b_qkv = np.asarray(b_qkv, np.float32)
    w_proj = np.asarray(w_proj, np.float32)

    b_eff = b_qkv + ln_b @ w_qkv
    w_eff = ln_w[:, None] * w_qkv
    w4 = w_eff.reshape(EMB, HEADS, HD, 3)
    b4 = b_eff.reshape(HEADS, HD, 3)
    wq = w4[..., 0] * SCALE
    wk = w4[..., 1]
    wv = w4[..., 2]
    bq = b4[..., 0] * SCALE
    bk = b4[..., 1]
    bv = b4[..., 2]

    def _dr(w):
        # [R, M] -> [R/2, 2M]: row 256c+128r+k -> (c*128+k, r*M+m)
        R, M = w.shape
        return np.ascontiguousarray(
            w.reshape(R // 256, 2, 128, M).transpose(0, 2, 1, 3).reshape(R // 2, 2 * M))

    in_maps = []
    for cid in range(N_CORES):
        bi, hg = divmod(cid, 2)
        hsl = slice(hg * HPC, (hg + 1) * HPC)
        in_maps.append({
            "x": np.ascontiguousarray(x[bi]),
            "wq": _dr(wq[:, hsl, :].reshape(EMB, QK_COLS)).astype(bf),
            "wk": _dr(wk[:, hsl, :].reshape(EMB, QK_COLS)).astype(bf),
            "wv": _dr(wv[:, hsl, :].reshape(EMB, QK_COLS)).astype(bf),
            "bqr": np.ascontiguousarray(
                bq[hsl].reshape(1, QK_COLS)).astype(bf),
            "bkr": np.ascontiguousarray(
                bk[hsl].reshape(1, QK_COLS)).astype(bf),
            "bv": np.ascontiguousarray(
                bv[hsl].reshape(1, QK_COLS)).astype(bf),
            "wp": _dr(w_proj[hg * QK_COLS:(hg + 1) * QK_COLS, :]).astype(bf),
        })
    return in_maps


def _gather(results, x, b_proj):
    b_proj = np.asarray(b_proj, np.float32)
    x = np.asarray(x, np.float32)
    out = np.empty((x.shape[0], N_TOK, EMB), np.float32)
    for bi in range(x.shape[0]):
        out[bi] = (results[2 * bi]["z"] + results[2 * bi + 1]["z"]
                   + b_proj[None, :] + x[bi])
    return out


def _run(inputs, **kw):
    in_maps = _prep_in_maps(**inputs)
    res = run_bass_kernel_spmd(_get_nc(), in_maps,
                               core_ids=list(range(N_CORES)), **kw)
    out = _gather(res.results, inputs["x"], inputs["b_proj"])
    return out, res


def kernel(**inputs):
    out, _ = _run(inputs)
    return out


# revision 21
# speedup vs baseline: 1.7335x; 1.0303x over previous
"""Trainium2 Bass kernel for a pre-LN multi-head attention block (v2).

Full-input contract: kernel(**inputs) takes the unsharded tensors from
setup_inputs() and returns the full [4, 2048, 1024] output.

Sharding: 8 cores = 4 batches x 2 head-groups (8 heads each).
Each core computes LayerNorm(x[b]) (replicated within the batch pair),
its 8 heads of QKV + attention, and a partial projection
(attn_out_part @ w_proj_rows).  Host sums the two partials per batch and
adds b_proj + residual.

Host-side algebraic folds (exact):
  - ln_w folded into w_qkv columns, ln_b folded into b_qkv
  - softmax scale (0.125, exact in fp32/bf16) folded into W_q / b_q

v2 design (vs v1): keep the PE tensor engine gaplessly busy so it holds
its high p-state, and split softmax-exp across the Scalar (true Exp) and
Vector (Schraudolph bit-trick exp -> bf16 via int16 bias/scale) engines:
  LN:    one-pass ACT normalize h = Identity(x*rstd + (-mean*rstd)),
         PE-transpose batched 4-per-PSUM-tile, copies split ACT/DVE
  QKV:   V per token tile + QK per 512-token chunk, pipelined with LN;
         biases folded in as rank-1 ones-row matmuls (PSUM accumulated)
  Attn:  per (h,q-chunk) unit: 16 ST matmuls pairwise-interleaved with
         16 AV matmuls of the previous unit; exp of k-tile c on ACT for
         c < N_ACT_EXP else DVE Schraudolph; softmax sums via ones-row
         65th V column; normalization deferred two units (stage A: copy
         sums row + fast reciprocal + casts; stage B: PE broadcast
         matmul + DVE multiply) so the PE never waits on it
  Proj:  interleaved one [128,512] PSUM group per unit once a q-chunk's
         outputs are complete
"""

import sys

sys.path.insert(0, "/opt/trn_rl_repo")

import numpy as np
import ml_dtypes

import concourse.bass as bass
from concourse import bacc
import concourse.tile as tile
from concourse import mybir
from concourse.bass_utils import run_bass_kernel_spmd
from concourse.masks import make_identity

EMB = 1024
HEADS = 16
HD = 64
SCALE = HD ** -0.5
N_TOK = 2048
N_CORES = 8
HPC = 8                 # heads per core
QK_COLS = HPC * HD      # 512
P = 128
NT = N_TOK // P         # 16 token tiles
EC = EMB // P           # 8 emb chunks
QCH = 4                 # q chunks of 512
NKT = 16                # k tiles of 128
NPAIR = HPC // 2        # 4 head-pair tiles

BF16 = mybir.dt.bfloat16
F32 = mybir.dt.float32
I16 = mybir.dt.int16
AF = mybir.ActivationFunctionType
ALU = mybir.AluOpType

# Schraudolph exp -> bf16 bit pattern: e^s ~= bitcast_bf16(int16(
#   s * 2^7/ln2 + (127*2^7 - 366000/2^16))).  The uniform bias component
# cancels in softmax normalization; only the ~1.5% mantissa-interp ripple
# survives, which the residual-dominated output dilutes ~13x.
# Both exp paths encode e^s/16 (ACT: Exp bias -4ln2 into fp8e4;
# DVE: bias lowered by 4*128 in the bf16 exponent field) so the softmax
# sums stay consistent; the /16 cancels in normalization.
A_SCHR = 184.6650292
B_SCHR = 15738.4153
N_ACT_EXP = 8           # k-tiles of exp on ACT (fp8 pairs); rest DVE (bf16)
FP8 = mybir.dt.float8e4
DRM = mybir.MatmulPerfMode.DoubleRow


def build_nc():
    nc = bacc.Bacc(trn_type="TRN2", target_bir_lowering=False)

    x_d = nc.dram_tensor("x", [N_TOK, EMB], F32, kind="ExternalInput")
    wq_d = nc.dram_tensor("wq", [EMB // 2, 2 * QK_COLS], BF16, kind="ExternalInput")
    wk_d = nc.dram_tensor("wk", [EMB // 2, 2 * QK_COLS], BF16, kind="ExternalInput")
    wv_d = nc.dram_tensor("wv", [EMB // 2, 2 * QK_COLS], BF16, kind="ExternalInput")
    bqr_d = nc.dram_tensor("bqr", [1, QK_COLS], BF16, kind="ExternalInput")
    bkr_d = nc.dram_tensor("bkr", [1, QK_COLS], BF16, kind="ExternalInput")
    bv_d = nc.dram_tensor("bv", [1, QK_COLS], BF16, kind="ExternalInput")
    wp_d = nc.dram_tensor("wp", [QK_COLS // 2, 2 * EMB], BF16, kind="ExternalInput")
    z_d = nc.dram_tensor("z", [N_TOK, EMB], F32, kind="ExternalOutput")

    with tile.TileContext(nc) as tc:
        _emit(nc, tc, x_d, wq_d, wk_d, wv_d, bqr_d, bkr_d, bv_d, wp_d, z_d)
    nc.finalize()
    return nc


def _emit(nc, tc, x_d, wq_d, wk_d, wv_d, bqr_d, bkr_d, bv_d, wp_d, z_d):
    from contextlib import ExitStack

    ctx = ExitStack()
    with ctx:
        consts = ctx.enter_context(tc.tile_pool(name="consts", bufs=1))
        persist = ctx.enter_context(tc.tile_pool(name="persist", bufs=1))

        ident = consts.tile([P, P], BF16, tag="ident", name="ident")
        make_identity(nc, ident)
        ones_row = consts.tile([1, 512], BF16, tag="ones_row", name="ones_row")
        nc.vector.memset(ones_row, 1.0)
        ones_sq = consts.tile([P, P], BF16, tag="ones_sq", name="ones_sq")
        nc.vector.memset(ones_sq, 1.0)
        eps_t = consts.tile([P, 1], F32, tag="eps", name="eps")
        nc.vector.memset(eps_t, 1e-5)

        bsh = consts.tile([P, 1], F32, tag="bsh", name="bsh")
        nc.vector.memset(bsh, -2.77258872)  # -4*ln2: ACT exp emits e^s/16

        bqr = consts.tile([1, QK_COLS], BF16, tag="bqr", name="bqr")
        nc.sync.dma_start(out=bqr, in_=bqr_d[:, :])
        bkr = consts.tile([1, QK_COLS], BF16, tag="bkr", name="bkr")
        nc.sync.dma_start(out=bkr, in_=bkr_d[:, :])
        bvt = consts.tile([1, QK_COLS], BF16, tag="bvt", name="bvt")
        nc.sync.dma_start(out=bvt, in_=bv_d[:, :])

        # weights arrive bf16 (prescaled by 2^6 / 2^5 host-side so fp8e4
        # normals cover them), cast on-chip to fp8 for DoubleRow matmuls
        wq_s = []
        wk_s = []
        wv_s = []
        for c in range(EC // 2):
            for lst, srcd, nm in ((wq_s, wq_d, "wq"), (wk_s, wk_d, "wk"),
                                  (wv_s, wv_d, "wv")):
                t = persist.tile([P, 2, QK_COLS], BF16, tag=f"{nm}{c}", name=f"{nm}{c}")
                nc.sync.dma_start(out=t, in_=srcd[c * P:(c + 1) * P, :].rearrange(
                    "p (r m) -> p r m", r=2))
                t8 = persist.tile([P, 2, QK_COLS], FP8, tag=f"{nm}8{c}", name=f"{nm}8{c}")
                nc.vector.tensor_copy(out=t8, in_=t)
                lst.append(t8)
        wp_s = []
        for i in range(2):
            t = persist.tile([P, 2, EMB], BF16, tag=f"wp{i}", name=f"wp{i}")
            nc.sync.dma_start(out=t, in_=wp_d[i * P:(i + 1) * P, :].rearrange(
                "p (r m) -> p r m", r=2))
            t8 = persist.tile([P, 2, EMB], FP8, tag=f"wp8{i}", name=f"wp8{i}")
            nc.vector.tensor_copy(out=t8, in_=t)
            wp_s.append(t8)

        # qt: one zero-padded tile per head -- the other head's 64 rows stay
        # zero so ST matmuls can run with the full K=128 kt stationary and
        # keep a single (128,128,512) matmul shape throughout attention
        # (alternating stationary shapes serializes LDWEIGHTS, ~1.5x cost).
        qt = [persist.tile([P, N_TOK], BF16, tag=f"qt{i}", name=f"qt{i}") for i in range(HPC)]
        for i in range(HPC):
            nc.vector.memset(qt[i], 0.0)
        kt = [persist.tile([P, N_TOK], BF16, tag=f"kt{i}", name=f"kt{i}") for i in range(NPAIR)]
        otn = [persist.tile([P, 2, N_TOK], FP8, tag=f"otn{i}", name=f"otn{i}") for i in range(2)]
        # vaug M padded 65 -> 128 with zeros (col 64 = ones for softmax sums)
        vaug = [persist.tile([P, 2, HPC, P], FP8, tag=f"vaug{i}", name=f"vaug{i}")
                for i in range(NT // 2)]
        for t in range(NT // 2):
            nc.vector.memset(vaug[t][:, :, :, HD:], 0.0)
            nc.vector.memset(vaug[t][:, :, :, HD:HD + 1], 1.0)

        # ---------- Phase 1+2: LayerNorm + transpose + V + QK ----------
        ht_ctx = ExitStack()
        ht_pool = ht_ctx.enter_context(tc.tile_pool(name="ht", bufs=1))
        ht = [ht_pool.tile([P, 2, N_TOK], FP8, tag=f"ht{e}", name=f"ht{e}") for e in range(EC // 2)]

        with tc.tile_pool(name="ln", bufs=3) as ln_pool, \
             tc.tile_pool(name="hp", bufs=2) as hp, \
             tc.tile_pool(name="lns", bufs=4) as lns, \
             tc.tile_pool(name="ps_tr", bufs=2, space="PSUM") as ps_tr, \
             tc.tile_pool(name="ps_qkv", bufs=3, space="PSUM") as ps_qkv:
            for t in range(NT):
                x_t = ln_pool.tile([P, EMB], F32, tag="x", name="x")
                nc.sync.dma_start(out=x_t, in_=x_d[t * P:(t + 1) * P, :])
                stats = lns.tile([P, 2, 6], F32, tag="stats", name="stats")
                nc.vector.bn_stats(out=stats[:, 0, :], in_=x_t[:, 0:512])
                nc.vector.bn_stats(out=stats[:, 1, :], in_=x_t[:, 512:1024])
                mv = lns.tile([P, 2], F32, tag="mv", name="mv")
                nc.vector.bn_aggr(out=mv, in_=stats)
                sd = lns.tile([P, 1], F32, tag="sd", name="sd")
                nc.scalar.activation(out=sd, in_=mv[:, 1:2], func=AF.Sqrt,
                                     bias=eps_t, scale=1.0)
                rstd = lns.tile([P, 1], F32, tag="rstd", name="rstd")
                nc.vector.reciprocal(out=rstd, in_=sd)
                nmrs = lns.tile([P, 1], F32, tag="nmrs", name="nmrs")
                nc.vector.scalar_tensor_tensor(
                    out=nmrs, in0=mv[:, 0:1], scalar=-1.0, in1=rstd,
                    op0=ALU.mult, op1=ALU.mult)
                h_bf = hp.tile([P, EMB], BF16, tag="h", name="h")
                nc.scalar.activation(out=h_bf, in_=x_t, func=AF.Identity,
                                     bias=nmrs, scale=rstd)
                # transpose: 4 chunks per PSUM tile, copy out 2x2 chunks
                for half in range(2):
                    pt = ps_tr.tile([P, 4, P], BF16, tag="tr", name="tr")
                    for j in range(4):
                        e = 4 * half + j
                        nc.tensor.transpose(pt[:, j, :],
                                            h_bf[:, e * P:(e + 1) * P], ident)
                    dst0 = ht[2 * half][:, :, t * P:(t + 1) * P]
                    dst1 = ht[2 * half + 1][:, :, t * P:(t + 1) * P]
                    if half == 0:
                        nc.scalar.copy(out=dst0, in_=pt[:, 0:2, :])
                        nc.scalar.copy(out=dst1, in_=pt[:, 2:4, :])
                    else:
                        nc.vector.tensor_copy(out=dst0, in_=pt[:, 0:2, :])
                        nc.vector.tensor_copy(out=dst1, in_=pt[:, 2:4, :])
                # V for this token tile (fp8 DoubleRow over emb pairs)
                pv = ps_qkv.tile([P, 512], F32, tag="qkv", name="qkv")
                for c in range(EC // 2):
                    nc.tensor.matmul(pv, lhsT=ht[c][:, :, t * P:(t + 1) * P],
                                     rhs=wv_s[c], start=(c == 0), stop=False,
                                     perf_mode=DRM)
                nc.tensor.matmul(pv, lhsT=ones_row[:, 0:P], rhs=bvt,
                                 start=False, stop=True)
                nc.vector.tensor_scalar_mul(
                    vaug[t // 2][:, t % 2, :, 0:HD],
                    pv.rearrange("p (h d) -> p h d", h=HPC), 0.03125)
                # QK for the completed 512-token chunk
                if t % 4 == 3:
                    n = t // 4
                    for m in range(NPAIR):
                        for w_s, brow, is_q in ((wq_s, bqr, True),
                                                (wk_s, bkr, False)):
                            pq = ps_qkv.tile([P, 512], F32, tag="qkv", name="qkv")
                            for c in range(EC // 2):
                                nc.tensor.matmul(
                                    pq, lhsT=w_s[c][:, :, m * P:(m + 1) * P],
                                    rhs=ht[c][:, :, n * 512:(n + 1) * 512],
                                    start=(c == 0), stop=False, perf_mode=DRM)
                            nc.tensor.matmul(pq, lhsT=brow[:, m * P:(m + 1) * P],
                                             rhs=ones_row, start=False, stop=True)
                            sl = slice(n * 512, (n + 1) * 512)
                            # undo the 2^6 fp8 weight prescale during copy-out
                            if is_q:
                                nc.scalar.mul(qt[2 * m][0:HD, sl],
                                              pq[0:HD, :], 0.015625)
                                nc.scalar.mul(qt[2 * m + 1][HD:P, sl],
                                              pq[HD:P, :], 0.015625)
                            else:
                                nc.scalar.mul(kt[m][:, sl], pq, 0.015625)

        ht_ctx.close()

        # ---------------- Phase 3: attention (+ proj interleaved) -------
        with tc.tile_pool(name="expp", bufs=2) as expp, \
             tc.tile_pool(name="att_sm", bufs=3) as att_sm, \
             tc.tile_pool(name="zst", bufs=3) as zst, \
             tc.tile_pool(name="ps_st", bufs=4, space="PSUM") as ps_st, \
             tc.tile_pool(name="ps_ot", bufs=2, space="PSUM") as ps_ot, \
             tc.tile_pool(name="ps_misc", bufs=2, space="PSUM") as ps_misc:

            def emit_proj_half(q, ti, half):
                """One projection PSUM group: tokens [128], emb cols [512]."""
                tt = q * 4 + ti
                pz = ps_misc.tile([P, 512], F32, tag="misc", name="pz")
                for cc in range(2):
                    nc.tensor.matmul(
                        pz, lhsT=otn[cc][:, :, tt * P:(tt + 1) * P],
                        rhs=wp_s[cc][:, :, half * 512:(half + 1) * 512],
                        start=(cc == 0), stop=(cc == 1), perf_mode=DRM)
                z_t = zst.tile([P, 512], F32, tag="z", name="z")
                # undo the 2^5 fp8 w_proj prescale
                nc.vector.tensor_scalar_mul(z_t, pz, 0.03125)
                nc.sync.dma_start(
                    out=z_d[tt * P:(tt + 1) * P, half * 512:(half + 1) * 512],
                    in_=z_t)

            # rec_pad slots: rows 1..127 must stay zero so the ones_sq
            # broadcast matmul (K=128, same shape as ST/AV) sees only row 0.
            rec_pads = []
            for j in range(3):
                rp = att_sm.tile([P, 512], BF16, tag=f"rec_pad{j}",
                                 name=f"rec_pad{j}", bufs=1)
                nc.vector.memset(rp, 0.0)
                rec_pads.append(rp)
            rp_idx = [0]

            def stage_a(u):
                """After AV stop: extract sums row, 1/Z, casts, raw copy."""
                q, h, pot = u
                srow = att_sm.tile([1, 512], F32, tag="srow", name="srow")
                nc.scalar.copy(out=srow, in_=pot[HD:HD + 1, :])
                rec = att_sm.tile([1, 512], F32, tag="rec", name="rec")
                nc.vector.reciprocal_approx_fast(out=rec, in_=srow)
                rec_pad = rec_pads[rp_idx[0] % 3]
                rp_idx[0] += 1
                nc.scalar.copy(out=rec_pad[0:1, :], in_=rec)
                ot_raw = att_sm.tile([HD, 512], BF16, tag="ot_raw", name="ot_raw")
                nc.scalar.copy(out=ot_raw, in_=pot[0:HD, :])
                return (q, h, rec_pad, ot_raw)

            def stage_b(u):
                """Two units later: broadcast 1/Z on PE, multiply into otn."""
                q, h, rec_pad, ot_raw = u
                pb = ps_misc.tile([P, 512], F32, tag="misc", name="pb")
                nc.tensor.matmul(pb, lhsT=ones_sq, rhs=rec_pad,
                                 start=True, stop=True)
                nc.vector.tensor_mul(
                    otn[h // 4][(h % 2) * HD:(h % 2) * HD + HD, (h // 2) % 2,
                                q * 512:(q + 1) * 512],
                    ot_raw, pb[0:HD, :])

            units = [(q, h) for q in range(QCH) for h in range(HPC)]
            av_prev = None      # (q, h, e_tiles) awaiting AV during this unit
            sa_queue = []       # stage-A results awaiting stage B (2-deep)
            proj_queue = []     # (q, ti, half) proj groups ready to emit

            def do_stage_b(sb):
                stage_b(sb)
                if sb[1] == HPC - 1:
                    proj_queue.extend(
                        (sb[0], ti, half) for ti in range(4) for half in range(2))

            for q, h in units:
                pair = h // 2
                # start-of-unit: one deferred normalize + one proj group.
                # both have inputs computed >= one full unit ago, so the PE
                # instructions here never wait on ACT/DVE.
                if len(sa_queue) >= 2:
                    do_stage_b(sa_queue.pop(0))
                if proj_queue:
                    emit_proj_half(*proj_queue.pop(0))
                # ACT k-tiles 0..7 as fp8 pairs (AV consumes via DoubleRow),
                # DVE k-tiles 8..15 as bf16 singles (AV consumes via bf16 mm)
                e8p = [expp.tile([P, 2, 512], FP8, tag=f"e8p{j}", name=f"e8p{j}")
                       for j in range(N_ACT_EXP // 2)]
                e16 = {c: expp.tile([P, 512], BF16, tag=f"e{c}", name=f"e{c}")
                       for c in range(N_ACT_EXP, NKT)}
                pot_prev = None
                if av_prev is not None:
                    pot_prev = ps_ot.tile([P, 512], F32, tag="ot", name="ot")

                def emit_av(c):
                    ph_, pe8_, pe16_ = av_prev[1], av_prev[2], av_prev[3]
                    if c < N_ACT_EXP:
                        if c % 2:
                            return  # consumed by the pair's DR matmul
                        nc.tensor.matmul(
                            pot_prev, lhsT=vaug[c // 2][:, :, ph_, :],
                            rhs=pe8_[c // 2], start=(c == 0), stop=False,
                            perf_mode=DRM)
                    else:
                        nc.tensor.matmul(
                            pot_prev, lhsT=vaug[c // 2][:, c % 2, ph_, :],
                            rhs=pe16_[c], start=False, stop=(c == NKT - 1))

                for c in range(NKT):
                    pst = ps_st.tile([P, 512], F32, tag="st", name="st")
                    nc.tensor.matmul(
                        pst,
                        lhsT=kt[pair][:, c * P:(c + 1) * P],
                        rhs=qt[h][:, q * 512:(q + 1) * 512],
                        start=True, stop=True)
                    if av_prev is not None:
                        emit_av(c)
                    if c < N_ACT_EXP:
                        nc.scalar.activation(out=e8p[c // 2][:, c % 2, :],
                                             in_=pst, func=AF.Exp, bias=bsh,
                                             scale=1.0)
                    else:
                        nc.vector.tensor_scalar(
                            out=e16[c].bitcast(I16), in0=pst,
                            scalar1=A_SCHR, scalar2=B_SCHR,
                            op0=ALU.mult, op1=ALU.add)
                # end-of-unit: AV of av_prev just completed -> stage A now
                if av_prev is not None:
                    sa_queue.append(stage_a((av_prev[0], av_prev[1], pot_prev)))
                av_prev = (q, h, e8p, e16)

            # drain: AV for the last unit, then remaining finalize stages
            if len(sa_queue) >= 2:
                do_stage_b(sa_queue.pop(0))
            pot_last = ps_ot.tile([P, 512], F32, tag="ot", name="ot")
            ph_, pe8_, pe16_ = av_prev[1], av_prev[2], av_prev[3]
            for j in range(N_ACT_EXP // 2):
                nc.tensor.matmul(pot_last, lhsT=vaug[j][:, :, ph_, :],
                                 rhs=pe8_[j], start=(j == 0), stop=False,
                                 perf_mode=DRM)
            for c in range(N_ACT_EXP, NKT):
                nc.tensor.matmul(
                    pot_last, lhsT=vaug[c // 2][:, c % 2, ph_, :],
                    rhs=pe16_[c], start=False, stop=(c == NKT - 1))
            sa_queue.append(stage_a((av_prev[0], av_prev[1], pot_last)))
            while sa_queue:
                do_stage_b(sa_queue.pop(0))
                if proj_queue:
                    emit_proj_half(*proj_queue.pop(0))
            for g in proj_queue:
                emit_proj_half(*g)
            proj_queue.clear()


_CACHE = {}


def _get_nc():
    if "nc" not in _CACHE:
        _CACHE["nc"] = build_nc()
    return _CACHE["nc"]


def _prep_in_maps(x, ln_w, ln_b, w_qkv, b_qkv, w_proj, b_proj):
    bf = ml_dtypes.bfloat16
    x = np.asarray(x, np.float32)
    ln_w = np.asarray(ln_w, np.float32)
    ln_b = np.asarray(ln_b, np.float32)
    w_qkv = np.asarray(w_qkv, np.float32)
    b_qkv = np.asarray(b_qkv, np.float32)
    w_proj = np.asarray(w_proj, np.float32)

    b_eff = b_qkv + ln_b @ w_qkv
    w_eff = ln_w[:, None] * w_qkv
    w4 = w_eff.reshape(EMB, HEADS, HD, 3)
    b4 = b_eff.reshape(HEADS, HD, 3)
    # sqrt(softmax scale) folded into both q and k; 2^6 (qk) / 2^5 (v, proj)
    # prescales lift the weights into fp8e4's normal range -- the kernel
    # multiplies the PSUM results by the inverse power of two on copy-out.
    sq_s = SCALE ** 0.5
    wq = w4[..., 0] * (sq_s * 64.0)
    wk = w4[..., 1] * (sq_s * 64.0)
    wv = w4[..., 2] * 32.0
    bq = b4[..., 0] * (sq_s * 64.0)
    bk = b4[..., 1] * (sq_s * 64.0)
    bv = b4[..., 2] * 32.0
    w_proj = w_proj * 32.0

    def _dr(w):
        # [R, M] -> [R/2, 2M]: row 256c+128r+k -> (c*128+k, r*M+m)
        R, M = w.shape
        return np.ascontiguousarray(
            w.reshape(R // 256, 2, 128, M).transpose(0, 2, 1, 3).reshape(R // 2, 2 * M))

    in_maps = []
    for cid in range(N_CORES):
        bi, hg = divmod(cid, 2)
        hsl = slice(hg * HPC, (hg + 1) * HPC)
        in_maps.append({
            "x": np.ascontiguousarray(x[bi]),
            "wq": _dr(wq[:, hsl, :].reshape(EMB, QK_COLS)).astype(bf),
            "wk": _dr(wk[:, hsl, :].reshape(EMB, QK_COLS)).astype(bf),
            "wv": _dr(wv[:, hsl, :].reshape(EMB, QK_COLS)).astype(bf),
            "bqr": np.ascontiguousarray(
                bq[hsl].reshape(1, QK_COLS)).astype(bf),
            "bkr": np.ascontiguousarray(
                bk[hsl].reshape(1, QK_COLS)).astype(bf),
            "bv": np.ascontiguousarray(
                bv[hsl].reshape(1, QK_COLS)).astype(bf),
            "wp": _dr(w_proj[hg * QK_COLS:(hg + 1) * QK_COLS, :]).astype(bf),
        })
    return in_maps


def _gather(results, x, b_proj):
    b_proj = np.asarray(b_proj, np.float32)
    x = np.asarray(x, np.float32)
    out = np.empty((x.shape[0], N_TOK, EMB), np.float32)
    for bi in range(x.shape[0]):
        out[bi] = (results[2 * bi]["z"] + results[2 * bi + 1]["z"]
                   + b_proj[None, :] + x[bi])
    return out


def _run(inputs, **kw):
    in_maps = _prep_in_maps(**inputs)
    res = run_bass_kernel_spmd(_get_nc(), in_maps,
                               core_ids=list(range(N_CORES)), **kw)
    out = _gather(res.results, inputs["x"], inputs["b_proj"])
    return out, res


def kernel(**inputs):
    out, _ = _run(inputs)
    return out


# revision 30
# speedup vs baseline: 1.8684x; 1.0778x over previous
"""Trainium2 Bass kernel for a pre-LN multi-head attention block (v2).

Full-input contract: kernel(**inputs) takes the unsharded tensors from
setup_inputs() and returns the full [4, 2048, 1024] output.

Sharding: 8 cores = 4 batches x 2 head-groups (8 heads each).
Each core computes LayerNorm(x[b]) (replicated within the batch pair),
its 8 heads of QKV + attention, and a partial projection
(attn_out_part @ w_proj_rows).  Host sums the two partials per batch and
adds b_proj + residual.

Host-side algebraic folds (exact):
  - ln_w folded into w_qkv columns, ln_b folded into b_qkv
  - softmax scale (0.125, exact in fp32/bf16) folded into W_q / b_q

v2 design (vs v1): keep the PE tensor engine gaplessly busy so it holds
its high p-state, and split softmax-exp across the Scalar (true Exp) and
Vector (Schraudolph bit-trick exp -> bf16 via int16 bias/scale) engines:
  LN:    one-pass ACT normalize h = Identity(x*rstd + (-mean*rstd)),
         PE-transpose batched 4-per-PSUM-tile, copies split ACT/DVE
  QKV:   V per token tile + QK per 512-token chunk, pipelined with LN;
         biases folded in as rank-1 ones-row matmuls (PSUM accumulated)
  Attn:  per (h,q-chunk) unit: 16 ST matmuls pairwise-interleaved with
         16 AV matmuls of the previous unit; exp of k-tile c on ACT for
         c < N_ACT_EXP else DVE Schraudolph; softmax sums via ones-row
         65th V column; normalization deferred two units (stage A: copy
         sums row + fast reciprocal + casts; stage B: PE broadcast
         matmul + DVE multiply) so the PE never waits on it
  Proj:  interleaved one [128,512] PSUM group per unit once a q-chunk's
         outputs are complete
"""

import sys

sys.path.insert(0, "/opt/trn_rl_repo")

import numpy as np
import ml_dtypes

import concourse.bass as bass
from concourse import bacc
import concourse.tile as tile
from concourse import mybir
from concourse.bass_utils import run_bass_kernel_spmd
from concourse.masks import make_identity

EMB = 1024
HEADS = 16
HD = 64
SCALE = HD ** -0.5
N_TOK = 2048
N_CORES = 8
HPC = 8                 # heads per core
QK_COLS = HPC * HD      # 512
P = 128
NT = N_TOK // P         # 16 token tiles
EC = EMB // P           # 8 emb chunks
QCH = 4                 # q chunks of 512
NKT = 16                # k tiles of 128
NPAIR = HPC // 2        # 4 head-pair tiles

BF16 = mybir.dt.bfloat16
F32 = mybir.dt.float32
I16 = mybir.dt.int16
AF = mybir.ActivationFunctionType
ALU = mybir.AluOpType

# Schraudolph exp -> bf16 bit pattern: e^s ~= bitcast_bf16(int16(
#   s * 2^7/ln2 + (127*2^7 - 366000/2^16))).  The uniform bias component
# cancels in softmax normalization; only the ~1.5% mantissa-interp ripple
# survives, which the residual-dominated output dilutes ~13x.
# Both exp paths encode e^s/16 (ACT: Exp bias -4ln2 into fp8e4;
# DVE: bias lowered by 4*128 in the bf16 exponent field) so the softmax
# sums stay consistent; the /16 cancels in normalization.
A_SCHR = 184.6650292
B_SCHR = 15738.4153
N_ACT_EXP = 8           # k-tiles of exp on ACT (fp8 pairs); rest DVE (bf16)
FP8 = mybir.dt.float8e4
DRM = mybir.MatmulPerfMode.DoubleRow


def build_nc():
    nc = bacc.Bacc(trn_type="TRN2", target_bir_lowering=False)

    x_d = nc.dram_tensor("x", [N_TOK, EMB], F32, kind="ExternalInput")
    wq_d = nc.dram_tensor("wq", [EMB // 2, 2 * QK_COLS], BF16, kind="ExternalInput")
    wk_d = nc.dram_tensor("wk", [EMB // 2, 2 * QK_COLS], BF16, kind="ExternalInput")
    wv_d = nc.dram_tensor("wv", [EMB // 2, 2 * QK_COLS], BF16, kind="ExternalInput")
    bqr_d = nc.dram_tensor("bqr", [1, QK_COLS], BF16, kind="ExternalInput")
    bkr_d = nc.dram_tensor("bkr", [1, QK_COLS], BF16, kind="ExternalInput")
    bv_d = nc.dram_tensor("bv", [1, QK_COLS], BF16, kind="ExternalInput")
    wp_d = nc.dram_tensor("wp", [QK_COLS // 2, 2 * EMB], BF16, kind="ExternalInput")
    z_d = nc.dram_tensor("z", [N_TOK, EMB], F32, kind="ExternalOutput")

    with tile.TileContext(nc) as tc:
        _emit(nc, tc, x_d, wq_d, wk_d, wv_d, bqr_d, bkr_d, bv_d, wp_d, z_d)
    nc.finalize()
    return nc


def _emit(nc, tc, x_d, wq_d, wk_d, wv_d, bqr_d, bkr_d, bv_d, wp_d, z_d):
    from contextlib import ExitStack

    ctx = ExitStack()
    with ctx:
        consts = ctx.enter_context(tc.tile_pool(name="consts", bufs=1))
        persist = ctx.enter_context(tc.tile_pool(name="persist", bufs=1))

        ident = consts.tile([P, P], BF16, tag="ident", name="ident")
        make_identity(nc, ident)
        ones_row = consts.tile([1, 512], BF16, tag="ones_row", name="ones_row")
        nc.vector.memset(ones_row, 1.0)
        ones_sq = consts.tile([P, P], BF16, tag="ones_sq", name="ones_sq")
        nc.vector.memset(ones_sq, 1.0)
        eps_t = consts.tile([P, 1], F32, tag="eps", name="eps")
        nc.vector.memset(eps_t, 1e-5)

        bsh = consts.tile([P, 1], F32, tag="bsh", name="bsh")
        nc.vector.memset(bsh, -2.77258872)  # -4*ln2: ACT exp emits e^s/16

        bqr = consts.tile([1, QK_COLS], BF16, tag="bqr", name="bqr")
        nc.sync.dma_start(out=bqr, in_=bqr_d[:, :])
        bkr = consts.tile([1, QK_COLS], BF16, tag="bkr", name="bkr")
        nc.sync.dma_start(out=bkr, in_=bkr_d[:, :])
        bvt = consts.tile([1, QK_COLS], BF16, tag="bvt", name="bvt")
        nc.sync.dma_start(out=bvt, in_=bv_d[:, :])

        def load_weights():
            # weights arrive bf16 (prescaled by 2^6 / 2^5 host-side so fp8e4
            # normals cover them), cast on-chip (ACT) to fp8 for
            # DoubleRow matmuls.  Called after the first x-tile DMAs are
            # queued so LayerNorm isn't stuck behind 3MB of weights.
            wq_s, wk_s, wv_s, wp_s = [], [], [], []
            for c in range(EC // 2):
                for lst, srcd, nm in ((wq_s, wq_d, "wq"), (wk_s, wk_d, "wk"),
                                      (wv_s, wv_d, "wv")):
                    t = persist.tile([P, 2, QK_COLS], BF16, tag=f"{nm}{c}", name=f"{nm}{c}")
                    nc.sync.dma_start(out=t, in_=srcd[c * P:(c + 1) * P, :].rearrange(
                        "p (r m) -> p r m", r=2))
                    t8 = persist.tile([P, 2, QK_COLS], FP8, tag=f"{nm}8{c}", name=f"{nm}8{c}")
                    nc.scalar.copy(out=t8, in_=t)
                    lst.append(t8)
            for i in range(2):
                t = persist.tile([P, 2, EMB], BF16, tag=f"wp{i}", name=f"wp{i}")
                nc.sync.dma_start(out=t, in_=wp_d[i * P:(i + 1) * P, :].rearrange(
                    "p (r m) -> p r m", r=2))
                t8 = persist.tile([P, 2, EMB], FP8, tag=f"wp8{i}", name=f"wp8{i}")
                nc.scalar.copy(out=t8, in_=t)
                wp_s.append(t8)
            return wq_s, wk_s, wv_s, wp_s

        # qt: one zero-padded tile per head -- the other head's 64 rows stay
        # zero so ST matmuls can run with the full K=128 kt stationary and
        # keep a single (128,128,512) matmul shape throughout attention
        # (alternating stationary shapes serializes LDWEIGHTS, ~1.5x cost).
        qt = [persist.tile([P, N_TOK], BF16, tag=f"qt{i}", name=f"qt{i}") for i in range(HPC)]
        for i in range(HPC):
            nc.vector.memset(qt[i], 0.0)
        kt = [persist.tile([P, N_TOK], BF16, tag=f"kt{i}", name=f"kt{i}") for i in range(NPAIR)]
        otn = [persist.tile([P, 2, N_TOK], FP8, tag=f"otn{i}", name=f"otn{i}") for i in range(2)]
        # vaug M padded 65 -> 128 with zeros (col 64 = ones for softmax sums)
        vaug = [persist.tile([P, 2, HPC, P], FP8, tag=f"vaug{i}", name=f"vaug{i}")
                for i in range(NT // 2)]
        for t in range(NT // 2):
            nc.vector.memset(vaug[t][:, :, :, HD:], 0.0)
            nc.vector.memset(vaug[t][:, :, :, HD:HD + 1], 1.0)

        # ---------- Phase 1+2: LayerNorm + transpose + V + QK ----------
        ht_ctx = ExitStack()
        ht_pool = ht_ctx.enter_context(tc.tile_pool(name="ht", bufs=1))
        ht = [ht_pool.tile([P, 2, N_TOK], FP8, tag=f"ht{e}", name=f"ht{e}") for e in range(EC // 2)]

        with tc.tile_pool(name="ln", bufs=5) as ln_pool, \
             tc.tile_pool(name="hp", bufs=2) as hp, \
             tc.tile_pool(name="lns", bufs=4) as lns, \
             tc.tile_pool(name="ps_tr", bufs=2, space="PSUM") as ps_tr, \
             tc.tile_pool(name="ps_qkv", bufs=3, space="PSUM") as ps_qkv:
            x_pre = []
            for t in range(4):
                x_t = ln_pool.tile([P, EMB], F32, tag="x", name="x")
                nc.sync.dma_start(out=x_t, in_=x_d[t * P:(t + 1) * P, :])
                x_pre.append(x_t)
            wq_s, wk_s, wv_s, wp_s = load_weights()
            for t in range(NT):
                if t < 4:
                    x_t = x_pre[t]
                else:
                    x_t = ln_pool.tile([P, EMB], F32, tag="x", name="x")
                    nc.sync.dma_start(out=x_t, in_=x_d[t * P:(t + 1) * P, :])
                stats = lns.tile([P, 2, 6], F32, tag="stats", name="stats")
                nc.vector.bn_stats(out=stats[:, 0, :], in_=x_t[:, 0:512])
                nc.vector.bn_stats(out=stats[:, 1, :], in_=x_t[:, 512:1024])
                mv = lns.tile([P, 2], F32, tag="mv", name="mv")
                nc.vector.bn_aggr(out=mv, in_=stats)
                sd = lns.tile([P, 1], F32, tag="sd", name="sd")
                nc.scalar.activation(out=sd, in_=mv[:, 1:2], func=AF.Sqrt,
                                     bias=eps_t, scale=1.0)
                rstd = lns.tile([P, 1], F32, tag="rstd", name="rstd")
                nc.vector.reciprocal(out=rstd, in_=sd)
                nmrs = lns.tile([P, 1], F32, tag="nmrs", name="nmrs")
                nc.vector.scalar_tensor_tensor(
                    out=nmrs, in0=mv[:, 0:1], scalar=-1.0, in1=rstd,
                    op0=ALU.mult, op1=ALU.mult)
                h_bf = hp.tile([P, EMB], BF16, tag="h", name="h")
                nc.scalar.activation(out=h_bf, in_=x_t, func=AF.Identity,
                                     bias=nmrs, scale=rstd)
                # transpose: 4 chunks per PSUM tile, copy out 2x2 chunks
                for half in range(2):
                    pt = ps_tr.tile([P, 4, P], BF16, tag="tr", name="tr")
                    for j in range(4):
                        e = 4 * half + j
                        nc.tensor.transpose(pt[:, j, :],
                                            h_bf[:, e * P:(e + 1) * P], ident)
                    dst0 = ht[2 * half][:, :, t * P:(t + 1) * P]
                    dst1 = ht[2 * half + 1][:, :, t * P:(t + 1) * P]
                    if half == 0:
                        nc.scalar.copy(out=dst0, in_=pt[:, 0:2, :])
                        nc.scalar.copy(out=dst1, in_=pt[:, 2:4, :])
                    else:
                        nc.vector.tensor_copy(out=dst0, in_=pt[:, 0:2, :])
                        nc.vector.tensor_copy(out=dst1, in_=pt[:, 2:4, :])
                # V for this token tile (fp8 DoubleRow over emb pairs)
                pv = ps_qkv.tile([P, 512], F32, tag="qkv", name="qkv")
                for c in range(EC // 2):
                    nc.tensor.matmul(pv, lhsT=ht[c][:, :, t * P:(t + 1) * P],
                                     rhs=wv_s[c], start=(c == 0), stop=False,
                                     perf_mode=DRM)
                nc.tensor.matmul(pv, lhsT=ones_row[:, 0:P], rhs=bvt,
                                 start=False, stop=True)
                nc.vector.tensor_scalar_mul(
                    vaug[t // 2][:, t % 2, :, 0:HD],
                    pv.rearrange("p (h d) -> p h d", h=HPC), 0.03125)
                # QK for the completed 512-token chunk
                if t % 4 == 3:
                    n = t // 4
                    for m in range(NPAIR):
                        for w_s, brow, is_q in ((wq_s, bqr, True),
                                                (wk_s, bkr, False)):
                            pq = ps_qkv.tile([P, 512], F32, tag="qkv", name="qkv")
                            for c in range(EC // 2):
                                nc.tensor.matmul(
                                    pq, lhsT=w_s[c][:, :, m * P:(m + 1) * P],
                                    rhs=ht[c][:, :, n * 512:(n + 1) * 512],
                                    start=(c == 0), stop=False, perf_mode=DRM)
                            nc.tensor.matmul(pq, lhsT=brow[:, m * P:(m + 1) * P],
                                             rhs=ones_row, start=False, stop=True)
                            sl = slice(n * 512, (n + 1) * 512)
                            # undo the 2^6 fp8 weight prescale during copy-out
                            if is_q:
                                nc.scalar.mul(qt[2 * m][0:HD, sl],
                                              pq[0:HD, :], 0.015625)
                                nc.scalar.mul(qt[2 * m + 1][HD:P, sl],
                                              pq[HD:P, :], 0.015625)
                            else:
                                nc.scalar.mul(kt[m][:, sl], pq, 0.015625)

        ht_ctx.close()

        # ---------------- Phase 3: attention (+ proj interleaved) -------
        with tc.tile_pool(name="expp", bufs=2) as expp, \
             tc.tile_pool(name="att_sm", bufs=3) as att_sm, \
             tc.tile_pool(name="zst", bufs=3) as zst, \
             tc.tile_pool(name="ps_st", bufs=4, space="PSUM") as ps_st, \
             tc.tile_pool(name="ps_ot", bufs=2, space="PSUM") as ps_ot, \
             tc.tile_pool(name="ps_misc", bufs=2, space="PSUM") as ps_misc:

            def emit_proj_half(q, ti, half):
                """One projection PSUM group: tokens [128], emb cols [512]."""
                tt = q * 4 + ti
                pz = ps_misc.tile([P, 512], F32, tag="misc", name="pz")
                for cc in range(2):
                    nc.tensor.matmul(
                        pz, lhsT=otn[cc][:, :, tt * P:(tt + 1) * P],
                        rhs=wp_s[cc][:, :, half * 512:(half + 1) * 512],
                        start=(cc == 0), stop=(cc == 1), perf_mode=DRM)
                z_t = zst.tile([P, 512], F32, tag="z", name="z")
                # undo the 2^5 fp8 w_proj prescale
                nc.vector.tensor_scalar_mul(z_t, pz, 0.03125)
                nc.sync.dma_start(
                    out=z_d[tt * P:(tt + 1) * P, half * 512:(half + 1) * 512],
                    in_=z_t)

            # rec_pad slots: rows 1..127 must stay zero so the ones_sq
            # broadcast matmul (K=128, same shape as ST/AV) sees only row 0.
            rec_pads = []
            for j in range(3):
                rp = att_sm.tile([P, 512], BF16, tag=f"rec_pad{j}",
                                 name=f"rec_pad{j}", bufs=1)
                nc.vector.memset(rp, 0.0)
                rec_pads.append(rp)
            rp_idx = [0]

            def stage_a(u):
                """After AV stop: 1/Z straight off the PSUM sums row, cast on
                gpsimd, raw attention rows to SBUF on ACT."""
                q, h, pot = u
                srow = att_sm.tile([1, 512], F32, tag="srow", name="srow")
                nc.scalar.copy(out=srow, in_=pot[HD:HD + 1, :])
                rec = att_sm.tile([1, 512], F32, tag="rec", name="rec")
                nc.vector.reciprocal_approx_fast(out=rec, in_=srow)
                rec_pad = rec_pads[rp_idx[0] % 3]
                rp_idx[0] += 1
                nc.scalar.copy(out=rec_pad[0:1, :], in_=rec)
                ot_raw = att_sm.tile([HD, 512], BF16, tag="ot_raw", name="ot_raw")
                nc.scalar.copy(out=ot_raw, in_=pot[0:HD, :])
                return (q, h, rec_pad, ot_raw)

            def stage_b(u):
                """Two units later: broadcast 1/Z on PE, multiply into otn."""
                q, h, rec_pad, ot_raw = u
                pb = ps_misc.tile([P, 512], F32, tag="misc", name="pb")
                nc.tensor.matmul(pb, lhsT=ones_sq, rhs=rec_pad,
                                 start=True, stop=True)
                nc.vector.tensor_mul(
                    otn[h // 4][(h % 2) * HD:(h % 2) * HD + HD, (h // 2) % 2,
                                q * 512:(q + 1) * 512],
                    ot_raw, pb[0:HD, :])

            units = [(q, h) for q in range(QCH) for h in range(HPC)]
            av_prev = None      # (q, h, e_tiles) awaiting AV during this unit
            sa_queue = []       # stage-A results awaiting stage B (2-deep)
            proj_queue = []     # (q, ti, half) proj groups ready to emit

            def do_stage_b(sb):
                stage_b(sb)
                if sb[1] == HPC - 1:
                    proj_queue.extend(
                        (sb[0], ti, half) for ti in range(4) for half in range(2))

            for q, h in units:
                pair = h // 2
                # start-of-unit: one deferred normalize + one proj group.
                # both have inputs computed >= one full unit ago, so the PE
                # instructions here never wait on ACT/DVE.
                if len(sa_queue) >= 2:
                    do_stage_b(sa_queue.pop(0))
                if proj_queue:
                    emit_proj_half(*proj_queue.pop(0))
                # ACT k-tiles 0..7 as fp8 pairs (AV consumes via DoubleRow),
                # DVE k-tiles 8..15 as bf16 singles (AV consumes via bf16 mm)
                e8p = [expp.tile([P, 2, 512], FP8, tag=f"e8p{j}", name=f"e8p{j}")
                       for j in range(N_ACT_EXP // 2)]
                e16 = {c: expp.tile([P, 512], BF16, tag=f"e{c}", name=f"e{c}")
                       for c in range(N_ACT_EXP, NKT)}
                pot_prev = None
                if av_prev is not None:
                    pot_prev = ps_ot.tile([P, 512], F32, tag="ot", name="ot")

                def emit_av(c):
                    ph_, pe8_, pe16_ = av_prev[1], av_prev[2], av_prev[3]
                    if c < N_ACT_EXP:
                        if c % 2:
                            return  # consumed by the pair's DR matmul
                        nc.tensor.matmul(
                            pot_prev, lhsT=vaug[c // 2][:, :, ph_, :],
                            rhs=pe8_[c // 2], start=(c == 0), stop=False,
                            perf_mode=DRM)
                    else:
                        nc.tensor.matmul(
                            pot_prev, lhsT=vaug[c // 2][:, c % 2, ph_, :],
                            rhs=pe16_[c], start=False, stop=(c == NKT - 1))

                for c in range(NKT):
                    pst = ps_st.tile([P, 512], F32, tag="st", name="st")
                    nc.tensor.matmul(
                        pst,
                        lhsT=kt[pair][:, c * P:(c + 1) * P],
                        rhs=qt[h][:, q * 512:(q + 1) * 512],
                        start=True, stop=True)
                    if av_prev is not None:
                        emit_av(c)
                    if c < N_ACT_EXP:
                        nc.scalar.activation(out=e8p[c // 2][:, c % 2, :],
                                             in_=pst, func=AF.Exp, bias=bsh,
                                             scale=1.0)
                    else:
                        nc.vector.tensor_scalar(
                            out=e16[c].bitcast(I16), in0=pst,
                            scalar1=A_SCHR, scalar2=B_SCHR,
                            op0=ALU.mult, op1=ALU.add)
                # end-of-unit: AV of av_prev just completed -> stage A now
                if av_prev is not None:
                    sa_queue.append(stage_a((av_prev[0], av_prev[1], pot_prev)))
                av_prev = (q, h, e8p, e16)

            # drain: AV for the last unit, then remaining finalize stages
            if len(sa_queue) >= 2:
                do_stage_b(sa_queue.pop(0))
            pot_last = ps_ot.tile([P, 512], F32, tag="ot", name="ot")
            ph_, pe8_, pe16_ = av_prev[1], av_prev[2], av_prev[3]
            for j in range(N_ACT_EXP // 2):
                nc.tensor.matmul(pot_last, lhsT=vaug[j][:, :, ph_, :],
                                 rhs=pe8_[j], start=(j == 0), stop=False,
                                 perf_mode=DRM)
            for c in range(N_ACT_EXP, NKT):
                nc.tensor.matmul(
                    pot_last, lhsT=vaug[c // 2][:, c % 2, ph_, :],
                    rhs=pe16_[c], start=False, stop=(c == NKT - 1))
            sa_queue.append(stage_a((av_prev[0], av_prev[1], pot_last)))
            while sa_queue:
                do_stage_b(sa_queue.pop(0))
                if proj_queue:
                    emit_proj_half(*proj_queue.pop(0))
            for g in proj_queue:
                emit_proj_half(*g)
            proj_queue.clear()


_CACHE = {}


def _get_nc():
    if "nc" not in _CACHE:
        _CACHE["nc"] = build_nc()
    return _CACHE["nc"]


def _prep_in_maps(x, ln_w, ln_b, w_qkv, b_qkv, w_proj, b_proj):
    bf = ml_dtypes.bfloat16
    x = np.asarray(x, np.float32)
    ln_w = np.asarray(ln_w, np.float32)
    ln_b = np.asarray(ln_b, np.float32)
    w_qkv = np.asarray(w_qkv, np.float32)
    b_qkv = np.asarray(b_qkv, np.float32)
    w_proj = np.asarray(w_proj, np.float32)

    b_eff = b_qkv + ln_b @ w_qkv
    w_eff = ln_w[:, None] * w_qkv
    w4 = w_eff.reshape(EMB, HEADS, HD, 3)
    b4 = b_eff.reshape(HEADS, HD, 3)
    # sqrt(softmax scale) folded into both q and k; 2^6 (qk) / 2^5 (v, proj)
    # prescales lift the weights into fp8e4's normal range -- the kernel
    # multiplies the PSUM results by the inverse power of two on copy-out.
    sq_s = SCALE ** 0.5
    wq = w4[..., 0] * (sq_s * 64.0)
    wk = w4[..., 1] * (sq_s * 64.0)
    wv = w4[..., 2] * 32.0
    bq = b4[..., 0] * (sq_s * 64.0)
    bk = b4[..., 1] * (sq_s * 64.0)
    bv = b4[..., 2] * 32.0
    w_proj = w_proj * 32.0

    def _dr(w):
        # [R, M] -> [R/2, 2M]: row 256c+128r+k -> (c*128+k, r*M+m)
        R, M = w.shape
        return np.ascontiguousarray(
            w.reshape(R // 256, 2, 128, M).transpose(0, 2, 1, 3).reshape(R // 2, 2 * M))

    in_maps = []
    for cid in range(N_CORES):
        bi, hg = divmod(cid, 2)
        hsl = slice(hg * HPC, (hg + 1) * HPC)
        in_maps.append({
            "x": np.ascontiguousarray(x[bi]),
            "wq": _dr(wq[:, hsl, :].reshape(EMB, QK_COLS)).astype(bf),
            "wk": _dr(wk[:, hsl, :].reshape(EMB, QK_COLS)).astype(bf),
            "wv": _dr(wv[:, hsl, :].reshape(EMB, QK_COLS)).astype(bf),
            "bqr": np.ascontiguousarray(
                bq[hsl].reshape(1, QK_COLS)).astype(bf),
            "bkr": np.ascontiguousarray(
                bk[hsl].reshape(1, QK_COLS)).astype(bf),
            "bv": np.ascontiguousarray(
                bv[hsl].reshape(1, QK_COLS)).astype(bf),
            "wp": _dr(w_proj[hg * QK_COLS:(hg + 1) * QK_COLS, :]).astype(bf),
        })
    return in_maps


def _gather(results, x, b_proj):
    b_proj = np.asarray(b_proj, np.float32)
    x = np.asarray(x, np.float32)
    out = np.empty((x.shape[0], N_TOK, EMB), np.float32)
    for bi in range(x.shape[0]):
        out[bi] = (results[2 * bi]["z"] + results[2 * bi + 1]["z"]
                   + b_proj[None, :] + x[bi])
    return out


def _run(inputs, **kw):
    in_maps = _prep_in_maps(**inputs)
    res = run_bass_kernel_spmd(_get_nc(), in_maps,
                               core_ids=list(range(N_CORES)), **kw)
    out = _gather(res.results, inputs["x"], inputs["b_proj"])
    return out, res


def kernel(**inputs):
    out, _ = _run(inputs)
    return out


# revision 33
# speedup vs baseline: 1.8859x; 1.0093x over previous
"""Trainium2 Bass kernel for a pre-LN multi-head attention block (v2).

Full-input contract: kernel(**inputs) takes the unsharded tensors from
setup_inputs() and returns the full [4, 2048, 1024] output.

Sharding: 8 cores = 4 batches x 2 head-groups (8 heads each).
Each core computes LayerNorm(x[b]) (replicated within the batch pair),
its 8 heads of QKV + attention, and a partial projection
(attn_out_part @ w_proj_rows).  Host sums the two partials per batch and
adds b_proj + residual.

Host-side algebraic folds (exact):
  - ln_w folded into w_qkv columns, ln_b folded into b_qkv
  - softmax scale (0.125, exact in fp32/bf16) folded into W_q / b_q

v2 design (vs v1): keep the PE tensor engine gaplessly busy so it holds
its high p-state, and split softmax-exp across the Scalar (true Exp) and
Vector (Schraudolph bit-trick exp -> bf16 via int16 bias/scale) engines:
  LN:    one-pass ACT normalize h = Identity(x*rstd + (-mean*rstd)),
         PE-transpose batched 4-per-PSUM-tile, copies split ACT/DVE
  QKV:   V per token tile + QK per 512-token chunk, pipelined with LN;
         biases folded in as rank-1 ones-row matmuls (PSUM accumulated)
  Attn:  per (h,q-chunk) unit: 16 ST matmuls pairwise-interleaved with
         16 AV matmuls of the previous unit; exp of k-tile c on ACT for
         c < N_ACT_EXP else DVE Schraudolph; softmax sums via ones-row
         65th V column; normalization deferred two units (stage A: copy
         sums row + fast reciprocal + casts; stage B: PE broadcast
         matmul + DVE multiply) so the PE never waits on it
  Proj:  interleaved one [128,512] PSUM group per unit once a q-chunk's
         outputs are complete
"""

import sys

sys.path.insert(0, "/opt/trn_rl_repo")

import numpy as np
import ml_dtypes

import concourse.bass as bass
from concourse import bacc
import concourse.tile as tile
from concourse import mybir
from concourse.bass_utils import run_bass_kernel_spmd
from concourse.masks import make_identity

EMB = 1024
HEADS = 16
HD = 64
SCALE = HD ** -0.5
N_TOK = 2048
N_CORES = 8
HPC = 8                 # heads per core
QK_COLS = HPC * HD      # 512
P = 128
NT = N_TOK // P         # 16 token tiles
EC = EMB // P           # 8 emb chunks
QCH = 4                 # q chunks of 512
NKT = 16                # k tiles of 128
NPAIR = HPC // 2        # 4 head-pair tiles

BF16 = mybir.dt.bfloat16
F32 = mybir.dt.float32
I16 = mybir.dt.int16
AF = mybir.ActivationFunctionType
ALU = mybir.AluOpType

# Schraudolph exp -> bf16 bit pattern: e^s ~= bitcast_bf16(int16(
#   s * 2^7/ln2 + (127*2^7 - 366000/2^16))).  The uniform bias component
# cancels in softmax normalization; only the ~1.5% mantissa-interp ripple
# survives, which the residual-dominated output dilutes ~13x.
# Both exp paths encode e^s/16 (ACT: Exp bias -4ln2 into fp8e4;
# DVE: bias lowered by 4*128 in the bf16 exponent field) so the softmax
# sums stay consistent; the /16 cancels in normalization.
# The Schraudolph multiplier A = 2^3/ln2 * (mantissa bits scale) for the
# fp8e4m3 target is folded into the Q weights host-side, so score PSUMs
# arrive as s*A_EXP.  DVE then needs only (add B, max 0) -> int8, which
# clamps the negative tail exactly; ACT's Exp absorbs 1/A_EXP into its
# scale immediate.  Both paths encode e^s/16 in fp8e4.
A_EXP = 11.5415643      # (2^7/ln2)/16
B_SCHR8 = 23.651        # 7*2^3 - 32 (the /16 shift) - 0.349 mantissa tune
N_ACT_EXP = 8           # k-tiles of exp on ACT; rest DVE (all fp8 pairs)
FP8 = mybir.dt.float8e4
I8 = mybir.dt.int8
DRM = mybir.MatmulPerfMode.DoubleRow


def build_nc():
    nc = bacc.Bacc(trn_type="TRN2", target_bir_lowering=False)

    x_d = nc.dram_tensor("x", [N_TOK, EMB], F32, kind="ExternalInput")
    wq_d = nc.dram_tensor("wq", [EMB // 2, 2 * QK_COLS], BF16, kind="ExternalInput")
    wk_d = nc.dram_tensor("wk", [EMB // 2, 2 * QK_COLS], BF16, kind="ExternalInput")
    wv_d = nc.dram_tensor("wv", [EMB // 2, 2 * QK_COLS], BF16, kind="ExternalInput")
    bqr_d = nc.dram_tensor("bqr", [1, QK_COLS], BF16, kind="ExternalInput")
    bkr_d = nc.dram_tensor("bkr", [1, QK_COLS], BF16, kind="ExternalInput")
    bv_d = nc.dram_tensor("bv", [1, QK_COLS], BF16, kind="ExternalInput")
    wp_d = nc.dram_tensor("wp", [QK_COLS // 2, 2 * EMB], BF16, kind="ExternalInput")
    z_d = nc.dram_tensor("z", [N_TOK, EMB], F32, kind="ExternalOutput")

    with tile.TileContext(nc) as tc:
        _emit(nc, tc, x_d, wq_d, wk_d, wv_d, bqr_d, bkr_d, bv_d, wp_d, z_d)
    nc.finalize()
    return nc


def _emit(nc, tc, x_d, wq_d, wk_d, wv_d, bqr_d, bkr_d, bv_d, wp_d, z_d):
    from contextlib import ExitStack

    ctx = ExitStack()
    with ctx:
        consts = ctx.enter_context(tc.tile_pool(name="consts", bufs=1))
        persist = ctx.enter_context(tc.tile_pool(name="persist", bufs=1))

        ident = consts.tile([P, P], BF16, tag="ident", name="ident")
        make_identity(nc, ident)
        ones_row = consts.tile([1, 512], BF16, tag="ones_row", name="ones_row")
        nc.vector.memset(ones_row, 1.0)
        ones_sq = consts.tile([P, P], BF16, tag="ones_sq", name="ones_sq")
        nc.vector.memset(ones_sq, 1.0)
        eps_t = consts.tile([P, 1], F32, tag="eps", name="eps")
        nc.vector.memset(eps_t, 1e-5)

        bsh = consts.tile([P, 1], F32, tag="bsh", name="bsh")
        nc.vector.memset(bsh, -2.77258872)  # -4*ln2: ACT exp emits e^s/16

        bqr = consts.tile([1, QK_COLS], BF16, tag="bqr", name="bqr")
        nc.sync.dma_start(out=bqr, in_=bqr_d[:, :])
        bkr = consts.tile([1, QK_COLS], BF16, tag="bkr", name="bkr")
        nc.sync.dma_start(out=bkr, in_=bkr_d[:, :])
        bvt = consts.tile([1, QK_COLS], BF16, tag="bvt", name="bvt")
        nc.sync.dma_start(out=bvt, in_=bv_d[:, :])

        def load_weights():
            # weights arrive bf16 (prescaled by 2^6 / 2^5 host-side so fp8e4
            # normals cover them), cast on-chip (ACT) to fp8 for
            # DoubleRow matmuls.  Called after the first x-tile DMAs are
            # queued so LayerNorm isn't stuck behind 3MB of weights.
            wq_s, wk_s, wv_s, wp_s = [], [], [], []
            for c in range(EC // 2):
                for lst, srcd, nm in ((wq_s, wq_d, "wq"), (wk_s, wk_d, "wk"),
                                      (wv_s, wv_d, "wv")):
                    t = persist.tile([P, 2, QK_COLS], BF16, tag=f"{nm}{c}", name=f"{nm}{c}")
                    nc.sync.dma_start(out=t, in_=srcd[c * P:(c + 1) * P, :].rearrange(
                        "p (r m) -> p r m", r=2))
                    t8 = persist.tile([P, 2, QK_COLS], FP8, tag=f"{nm}8{c}", name=f"{nm}8{c}")
                    nc.scalar.copy(out=t8, in_=t)
                    lst.append(t8)
            for i in range(2):
                t = persist.tile([P, 2, EMB], BF16, tag=f"wp{i}", name=f"wp{i}")
                nc.sync.dma_start(out=t, in_=wp_d[i * P:(i + 1) * P, :].rearrange(
                    "p (r m) -> p r m", r=2))
                t8 = persist.tile([P, 2, EMB], FP8, tag=f"wp8{i}", name=f"wp8{i}")
                nc.scalar.copy(out=t8, in_=t)
                wp_s.append(t8)
            return wq_s, wk_s, wv_s, wp_s

        # qt: one zero-padded tile per head -- the other head's 64 rows stay
        # zero so ST matmuls can run with the full K=128 kt stationary and
        # keep a single (128,128,512) matmul shape throughout attention
        # (alternating stationary shapes serializes LDWEIGHTS, ~1.5x cost).
        qt = [persist.tile([P, N_TOK], BF16, tag=f"qt{i}", name=f"qt{i}") for i in range(HPC)]
        for i in range(HPC):
            nc.vector.memset(qt[i], 0.0)
        kt = [persist.tile([P, N_TOK], BF16, tag=f"kt{i}", name=f"kt{i}") for i in range(NPAIR)]
        otn = [persist.tile([P, 2, N_TOK], FP8, tag=f"otn{i}", name=f"otn{i}") for i in range(2)]
        # vaug M padded 65 -> 128 with zeros (col 64 = ones for softmax sums)
        vaug = [persist.tile([P, 2, HPC, P], FP8, tag=f"vaug{i}", name=f"vaug{i}")
                for i in range(NT // 2)]
        for t in range(NT // 2):
            nc.vector.memset(vaug[t][:, :, :, HD:], 0.0)
            nc.vector.memset(vaug[t][:, :, :, HD:HD + 1], 1.0)

        # ---------- Phase 1+2: LayerNorm + transpose + V + QK ----------
        ht_ctx = ExitStack()
        ht_pool = ht_ctx.enter_context(tc.tile_pool(name="ht", bufs=1))
        ht = [ht_pool.tile([P, 2, N_TOK], FP8, tag=f"ht{e}", name=f"ht{e}") for e in range(EC // 2)]

        with tc.tile_pool(name="ln", bufs=5) as ln_pool, \
             tc.tile_pool(name="hp", bufs=2) as hp, \
             tc.tile_pool(name="lns", bufs=4) as lns, \
             tc.tile_pool(name="ps_tr", bufs=2, space="PSUM") as ps_tr, \
             tc.tile_pool(name="ps_qkv", bufs=3, space="PSUM") as ps_qkv:
            x_pre = []
            for t in range(4):
                x_t = ln_pool.tile([P, EMB], F32, tag="x", name="x")
                nc.sync.dma_start(out=x_t, in_=x_d[t * P:(t + 1) * P, :])
                x_pre.append(x_t)
            wq_s, wk_s, wv_s, wp_s = load_weights()
            for t in range(NT):
                if t < 4:
                    x_t = x_pre[t]
                else:
                    x_t = ln_pool.tile([P, EMB], F32, tag="x", name="x")
                    nc.sync.dma_start(out=x_t, in_=x_d[t * P:(t + 1) * P, :])
                stats = lns.tile([P, 2, 6], F32, tag="stats", name="stats")
                nc.vector.bn_stats(out=stats[:, 0, :], in_=x_t[:, 0:512])
                nc.vector.bn_stats(out=stats[:, 1, :], in_=x_t[:, 512:1024])
                mv = lns.tile([P, 2], F32, tag="mv", name="mv")
                nc.vector.bn_aggr(out=mv, in_=stats)
                sd = lns.tile([P, 1], F32, tag="sd", name="sd")
                nc.scalar.activation(out=sd, in_=mv[:, 1:2], func=AF.Sqrt,
                                     bias=eps_t, scale=1.0)
                rstd = lns.tile([P, 1], F32, tag="rstd", name="rstd")
                nc.vector.reciprocal(out=rstd, in_=sd)
                nmrs = lns.tile([P, 1], F32, tag="nmrs", name="nmrs")
                nc.vector.scalar_tensor_tensor(
                    out=nmrs, in0=mv[:, 0:1], scalar=-1.0, in1=rstd,
                    op0=ALU.mult, op1=ALU.mult)
                h_bf = hp.tile([P, EMB], BF16, tag="h", name="h")
                nc.scalar.activation(out=h_bf, in_=x_t, func=AF.Identity,
                                     bias=nmrs, scale=rstd)
                # transpose: 4 chunks per PSUM tile, copy out 2x2 chunks
                for half in range(2):
                    pt = ps_tr.tile([P, 4, P], BF16, tag="tr", name="tr")
                    for j in range(4):
                        e = 4 * half + j
                        nc.tensor.transpose(pt[:, j, :],
                                            h_bf[:, e * P:(e + 1) * P], ident)
                    dst0 = ht[2 * half][:, :, t * P:(t + 1) * P]
                    dst1 = ht[2 * half + 1][:, :, t * P:(t + 1) * P]
                    if half == 0:
                        nc.scalar.copy(out=dst0, in_=pt[:, 0:2, :])
                        nc.scalar.copy(out=dst1, in_=pt[:, 2:4, :])
                    else:
                        nc.vector.tensor_copy(out=dst0, in_=pt[:, 0:2, :])
                        nc.vector.tensor_copy(out=dst1, in_=pt[:, 2:4, :])
                # V for this token tile (fp8 DoubleRow over emb pairs)
                pv = ps_qkv.tile([P, 512], F32, tag="qkv", name="qkv")
                for c in range(EC // 2):
                    nc.tensor.matmul(pv, lhsT=ht[c][:, :, t * P:(t + 1) * P],
                                     rhs=wv_s[c], start=(c == 0), stop=False,
                                     perf_mode=DRM)
                nc.tensor.matmul(pv, lhsT=ones_row[:, 0:P], rhs=bvt,
                                 start=False, stop=True)
                nc.vector.tensor_scalar_mul(
                    vaug[t // 2][:, t % 2, :, 0:HD],
                    pv.rearrange("p (h d) -> p h d", h=HPC), 0.03125)
                # QK for the completed 512-token chunk
                if t % 4 == 3:
                    n = t // 4
                    for m in range(NPAIR):
                        for w_s, brow, is_q in ((wq_s, bqr, True),
                                                (wk_s, bkr, False)):
                            pq = ps_qkv.tile([P, 512], F32, tag="qkv", name="qkv")
                            for c in range(EC // 2):
                                nc.tensor.matmul(
                                    pq, lhsT=w_s[c][:, :, m * P:(m + 1) * P],
                                    rhs=ht[c][:, :, n * 512:(n + 1) * 512],
                                    start=(c == 0), stop=False, perf_mode=DRM)
                            nc.tensor.matmul(pq, lhsT=brow[:, m * P:(m + 1) * P],
                                             rhs=ones_row, start=False, stop=True)
                            sl = slice(n * 512, (n + 1) * 512)
                            # undo the 2^6 fp8 weight prescale during copy-out
                            if is_q:
                                nc.scalar.mul(qt[2 * m][0:HD, sl],
                                              pq[0:HD, :], 0.015625)
                                nc.scalar.mul(qt[2 * m + 1][HD:P, sl],
                                              pq[HD:P, :], 0.015625)
                            else:
                                nc.scalar.mul(kt[m][:, sl], pq, 0.015625)

        ht_ctx.close()

        # ---------------- Phase 3: attention (+ proj interleaved) -------
        with tc.tile_pool(name="expp", bufs=2) as expp, \
             tc.tile_pool(name="att_sm", bufs=3) as att_sm, \
             tc.tile_pool(name="zst", bufs=3) as zst, \
             tc.tile_pool(name="ps_st", bufs=4, space="PSUM") as ps_st, \
             tc.tile_pool(name="ps_ot", bufs=2, space="PSUM") as ps_ot, \
             tc.tile_pool(name="ps_misc", bufs=2, space="PSUM") as ps_misc:

            def emit_proj_half(q, ti, half):
                """One projection PSUM group: tokens [128], emb cols [512]."""
                tt = q * 4 + ti
                pz = ps_misc.tile([P, 512], F32, tag="misc", name="pz")
                for cc in range(2):
                    nc.tensor.matmul(
                        pz, lhsT=otn[cc][:, :, tt * P:(tt + 1) * P],
                        rhs=wp_s[cc][:, :, half * 512:(half + 1) * 512],
                        start=(cc == 0), stop=(cc == 1), perf_mode=DRM)
                z_t = zst.tile([P, 512], F32, tag="z", name="z")
                # undo the 2^5 fp8 w_proj prescale
                nc.vector.tensor_scalar_mul(z_t, pz, 0.03125)
                nc.sync.dma_start(
                    out=z_d[tt * P:(tt + 1) * P, half * 512:(half + 1) * 512],
                    in_=z_t)

            # rec_pad slots: rows 1..127 must stay zero so the ones_sq
            # broadcast matmul (K=128, same shape as ST/AV) sees only row 0.
            rec_pads = []
            for j in range(3):
                rp = att_sm.tile([P, 512], BF16, tag=f"rec_pad{j}",
                                 name=f"rec_pad{j}", bufs=1)
                nc.vector.memset(rp, 0.0)
                rec_pads.append(rp)
            rp_idx = [0]

            def stage_a(u):
                """After AV stop: 1/Z straight off the PSUM sums row, cast on
                gpsimd, raw attention rows to SBUF on ACT."""
                q, h, pot = u
                srow = att_sm.tile([1, 512], F32, tag="srow", name="srow")
                nc.scalar.copy(out=srow, in_=pot[HD:HD + 1, :])
                rec = att_sm.tile([1, 512], F32, tag="rec", name="rec")
                nc.vector.reciprocal_approx_fast(out=rec, in_=srow)
                rec_pad = rec_pads[rp_idx[0] % 3]
                rp_idx[0] += 1
                nc.scalar.copy(out=rec_pad[0:1, :], in_=rec)
                ot_raw = att_sm.tile([HD, 512], BF16, tag="ot_raw", name="ot_raw")
                nc.scalar.copy(out=ot_raw, in_=pot[0:HD, :])
                return (q, h, rec_pad, ot_raw)

            def stage_b(u):
                """Two units later: broadcast 1/Z on PE, multiply into otn."""
                q, h, rec_pad, ot_raw = u
                pb = ps_misc.tile([P, 512], F32, tag="misc", name="pb")
                nc.tensor.matmul(pb, lhsT=ones_sq, rhs=rec_pad,
                                 start=True, stop=True)
                nc.vector.tensor_mul(
                    otn[h // 4][(h % 2) * HD:(h % 2) * HD + HD, (h // 2) % 2,
                                q * 512:(q + 1) * 512],
                    ot_raw, pb[0:HD, :])

            units = [(q, h) for q in range(QCH) for h in range(HPC)]
            av_prev = None      # (q, h, e_tiles) awaiting AV during this unit
            sa_queue = []       # stage-A results awaiting stage B (2-deep)
            proj_queue = []     # (q, ti, half) proj groups ready to emit

            def do_stage_b(sb):
                stage_b(sb)
                if sb[1] == HPC - 1:
                    proj_queue.extend(
                        (sb[0], ti, half) for ti in range(4) for half in range(2))

            for q, h in units:
                pair = h // 2
                # start-of-unit: one deferred normalize + one proj group.
                # both have inputs computed >= one full unit ago, so the PE
                # instructions here never wait on ACT/DVE.
                if len(sa_queue) >= 2:
                    do_stage_b(sa_queue.pop(0))
                if proj_queue:
                    emit_proj_half(*proj_queue.pop(0))
                # ACT k-tiles 0..7 as fp8 pairs (AV consumes via DoubleRow),
                # DVE k-tiles 8..15 as bf16 singles (AV consumes via bf16 mm)
                e8p = [expp.tile([P, 2, 512], FP8, tag=f"e8p{j}", name=f"e8p{j}")
                       for j in range(NKT // 2)]
                pot_prev = None
                if av_prev is not None:
                    pot_prev = ps_ot.tile([P, 512], F32, tag="ot", name="ot")

                def emit_av(c):
                    if c % 2:
                        return  # consumed by the pair's DR matmul
                    ph_, pe8_ = av_prev[1], av_prev[2]
                    nc.tensor.matmul(
                        pot_prev, lhsT=vaug[c // 2][:, :, ph_, :],
                        rhs=pe8_[c // 2], start=(c == 0),
                        stop=(c == NKT - 2), perf_mode=DRM)

                for c in range(NKT):
                    pst = ps_st.tile([P, 512], F32, tag="st", name="st")
                    nc.tensor.matmul(
                        pst,
                        lhsT=kt[pair][:, c * P:(c + 1) * P],
                        rhs=qt[h][:, q * 512:(q + 1) * 512],
                        start=True, stop=True)
                    if av_prev is not None:
                        emit_av(c)
                    if c < N_ACT_EXP:
                        nc.scalar.activation(out=e8p[c // 2][:, c % 2, :],
                                             in_=pst, func=AF.Exp, bias=bsh,
                                             scale=1.0 / A_EXP)
                    else:
                        nc.vector.tensor_scalar(
                            out=e8p[c // 2][:, c % 2, :].bitcast(I8), in0=pst,
                            scalar1=B_SCHR8, scalar2=0.0,
                            op0=ALU.add, op1=ALU.max)
                # end-of-unit: AV of av_prev just completed -> stage A now
                if av_prev is not None:
                    sa_queue.append(stage_a((av_prev[0], av_prev[1], pot_prev)))
                av_prev = (q, h, e8p)

            # drain: AV for the last unit, then remaining finalize stages
            if len(sa_queue) >= 2:
                do_stage_b(sa_queue.pop(0))
            pot_last = ps_ot.tile([P, 512], F32, tag="ot", name="ot")
            ph_, pe8_ = av_prev[1], av_prev[2]
            for j in range(NKT // 2):
                nc.tensor.matmul(pot_last, lhsT=vaug[j][:, :, ph_, :],
                                 rhs=pe8_[j], start=(j == 0),
                                 stop=(j == NKT // 2 - 1), perf_mode=DRM)
            sa_queue.append(stage_a((av_prev[0], av_prev[1], pot_last)))
            while sa_queue:
                do_stage_b(sa_queue.pop(0))
                if proj_queue:
                    emit_proj_half(*proj_queue.pop(0))
            for g in proj_queue:
                emit_proj_half(*g)
            proj_queue.clear()


_CACHE = {}


def _get_nc():
    if "nc" not in _CACHE:
        _CACHE["nc"] = build_nc()
    return _CACHE["nc"]


def _prep_in_maps(x, ln_w, ln_b, w_qkv, b_qkv, w_proj, b_proj):
    bf = ml_dtypes.bfloat16
    x = np.asarray(x, np.float32)
    ln_w = np.asarray(ln_w, np.float32)
    ln_b = np.asarray(ln_b, np.float32)
    w_qkv = np.asarray(w_qkv, np.float32)
    b_qkv = np.asarray(b_qkv, np.float32)
    w_proj = np.asarray(w_proj, np.float32)

    b_eff = b_qkv + ln_b @ w_qkv
    w_eff = ln_w[:, None] * w_qkv
    w4 = w_eff.reshape(EMB, HEADS, HD, 3)
    b4 = b_eff.reshape(HEADS, HD, 3)
    # sqrt(softmax scale) folded into both q and k; 2^6 (qk) / 2^5 (v, proj)
    # prescales lift the weights into fp8e4's normal range -- the kernel
    # multiplies the PSUM results by the inverse power of two on copy-out.
    sq_s = SCALE ** 0.5
    wq = w4[..., 0] * (sq_s * 64.0 * A_EXP)
    wk = w4[..., 1] * (sq_s * 64.0)
    wv = w4[..., 2] * 32.0
    bq = b4[..., 0] * (sq_s * 64.0 * A_EXP)
    bk = b4[..., 1] * (sq_s * 64.0)
    bv = b4[..., 2] * 32.0
    w_proj = w_proj * 32.0

    def _dr(w):
        # [R, M] -> [R/2, 2M]: row 256c+128r+k -> (c*128+k, r*M+m)
        R, M = w.shape
        return np.ascontiguousarray(
            w.reshape(R // 256, 2, 128, M).transpose(0, 2, 1, 3).reshape(R // 2, 2 * M))

    in_maps = []
    for cid in range(N_CORES):
        bi, hg = divmod(cid, 2)
        hsl = slice(hg * HPC, (hg + 1) * HPC)
        in_maps.append({
            "x": np.ascontiguousarray(x[bi]),
            "wq": _dr(wq[:, hsl, :].reshape(EMB, QK_COLS)).astype(bf),
            "wk": _dr(wk[:, hsl, :].reshape(EMB, QK_COLS)).astype(bf),
            "wv": _dr(wv[:, hsl, :].reshape(EMB, QK_COLS)).astype(bf),
            "bqr": np.ascontiguousarray(
                bq[hsl].reshape(1, QK_COLS)).astype(bf),
            "bkr": np.ascontiguousarray(
                bk[hsl].reshape(1, QK_COLS)).astype(bf),
            "bv": np.ascontiguousarray(
                bv[hsl].reshape(1, QK_COLS)).astype(bf),
            "wp": _dr(w_proj[hg * QK_COLS:(hg + 1) * QK_COLS, :]).astype(bf),
        })
    return in_maps


def _gather(results, x, b_proj):
    b_proj = np.asarray(b_proj, np.float32)
    x = np.asarray(x, np.float32)
    out = np.empty((x.shape[0], N_TOK, EMB), np.float32)
    for bi in range(x.shape[0]):
        out[bi] = (results[2 * bi]["z"] + results[2 * bi + 1]["z"]
                   + b_proj[None, :] + x[bi])
    return out


def _run(inputs, **kw):
    in_maps = _prep_in_maps(**inputs)
    res = run_bass_kernel_spmd(_get_nc(), in_maps,
                               core_ids=list(range(N_CORES)), **kw)
    out = _gather(res.results, inputs["x"], inputs["b_proj"])
    return out, res


def kernel(**inputs):
    out, _ = _run(inputs)
    return out


# revision 35
# speedup vs baseline: 1.8931x; 1.0038x over previous
"""Trainium2 Bass kernel for a pre-LN multi-head attention block (v2).

Full-input contract: kernel(**inputs) takes the unsharded tensors from
setup_inputs() and returns the full [4, 2048, 1024] output.

Sharding: 8 cores = 4 batches x 2 head-groups (8 heads each).
Each core computes LayerNorm(x[b]) (replicated within the batch pair),
its 8 heads of QKV + attention, and a partial projection
(attn_out_part @ w_proj_rows).  Host sums the two partials per batch and
adds b_proj + residual.

Host-side algebraic folds (exact):
  - ln_w folded into w_qkv columns, ln_b folded into b_qkv
  - softmax scale (0.125, exact in fp32/bf16) folded into W_q / b_q

v2 design (vs v1): keep the PE tensor engine gaplessly busy so it holds
its high p-state, and split softmax-exp across the Scalar (true Exp) and
Vector (Schraudolph bit-trick exp -> bf16 via int16 bias/scale) engines:
  LN:    one-pass ACT normalize h = Identity(x*rstd + (-mean*rstd)),
         PE-transpose batched 4-per-PSUM-tile, copies split ACT/DVE
  QKV:   V per token tile + QK per 512-token chunk, pipelined with LN;
         biases folded in as rank-1 ones-row matmuls (PSUM accumulated)
  Attn:  per (h,q-chunk) unit: 16 ST matmuls pairwise-interleaved with
         16 AV matmuls of the previous unit; exp of k-tile c on ACT for
         c < N_ACT_EXP else DVE Schraudolph; softmax sums via ones-row
         65th V column; normalization deferred two units (stage A: copy
         sums row + fast reciprocal + casts; stage B: PE broadcast
         matmul + DVE multiply) so the PE never waits on it
  Proj:  interleaved one [128,512] PSUM group per unit once a q-chunk's
         outputs are complete
"""

import sys

sys.path.insert(0, "/opt/trn_rl_repo")

import numpy as np
import ml_dtypes

import concourse.bass as bass
from concourse import bacc
import concourse.tile as tile
from concourse import mybir
from concourse.bass_utils import run_bass_kernel_spmd
from concourse.masks import make_identity

EMB = 1024
HEADS = 16
HD = 64
SCALE = HD ** -0.5
N_TOK = 2048
N_CORES = 8
HPC = 8                 # heads per core
QK_COLS = HPC * HD      # 512
P = 128
NT = N_TOK // P         # 16 token tiles
EC = EMB // P           # 8 emb chunks
QCH = 4                 # q chunks of 512
NKT = 16                # k tiles of 128
NPAIR = HPC // 2        # 4 head-pair tiles

BF16 = mybir.dt.bfloat16
F32 = mybir.dt.float32
I16 = mybir.dt.int16
AF = mybir.ActivationFunctionType
ALU = mybir.AluOpType

# Schraudolph exp -> bf16 bit pattern: e^s ~= bitcast_bf16(int16(
#   s * 2^7/ln2 + (127*2^7 - 366000/2^16))).  The uniform bias component
# cancels in softmax normalization; only the ~1.5% mantissa-interp ripple
# survives, which the residual-dominated output dilutes ~13x.
# Both exp paths encode e^s/16 (ACT: Exp bias -4ln2 into fp8e4;
# DVE: bias lowered by 4*128 in the bf16 exponent field) so the softmax
# sums stay consistent; the /16 cancels in normalization.
# The Schraudolph multiplier A = 2^3/ln2 * (mantissa bits scale) for the
# fp8e4m3 target is folded into the Q weights host-side, so score PSUMs
# arrive as s*A_EXP.  DVE then needs only (add B, max 0) -> int8, which
# clamps the negative tail exactly; ACT's Exp absorbs 1/A_EXP into its
# scale immediate.  Both paths encode e^s/16 in fp8e4.
A_EXP = 11.5415643      # (2^7/ln2)/16
B_SCHR8 = 23.651        # 7*2^3 - 32 (the /16 shift) - 0.349 mantissa tune
N_ACT_EXP = 8           # k-tiles of exp on ACT; rest DVE (all fp8 pairs)
FP8 = mybir.dt.float8e4
I8 = mybir.dt.int8
DRM = mybir.MatmulPerfMode.DoubleRow


def build_nc():
    nc = bacc.Bacc(trn_type="TRN2", target_bir_lowering=False)

    x_d = nc.dram_tensor("x", [N_TOK, EMB], F32, kind="ExternalInput")
    wq_d = nc.dram_tensor("wq", [EMB // 2, 2 * QK_COLS], BF16, kind="ExternalInput")
    wk_d = nc.dram_tensor("wk", [EMB // 2, 2 * QK_COLS], BF16, kind="ExternalInput")
    wv_d = nc.dram_tensor("wv", [EMB // 2, 2 * QK_COLS], BF16, kind="ExternalInput")
    bqr_d = nc.dram_tensor("bqr", [1, QK_COLS], BF16, kind="ExternalInput")
    bkr_d = nc.dram_tensor("bkr", [1, QK_COLS], BF16, kind="ExternalInput")
    bv_d = nc.dram_tensor("bv", [1, QK_COLS], BF16, kind="ExternalInput")
    wp_d = nc.dram_tensor("wp", [QK_COLS // 2, 2 * EMB], BF16, kind="ExternalInput")
    z_d = nc.dram_tensor("z", [N_TOK, EMB], F32, kind="ExternalOutput")

    with tile.TileContext(nc) as tc:
        _emit(nc, tc, x_d, wq_d, wk_d, wv_d, bqr_d, bkr_d, bv_d, wp_d, z_d)
    nc.finalize()
    return nc


def _emit(nc, tc, x_d, wq_d, wk_d, wv_d, bqr_d, bkr_d, bv_d, wp_d, z_d):
    from contextlib import ExitStack

    ctx = ExitStack()
    with ctx:
        consts = ctx.enter_context(tc.tile_pool(name="consts", bufs=1))
        persist = ctx.enter_context(tc.tile_pool(name="persist", bufs=1))

        ident = consts.tile([P, P], BF16, tag="ident", name="ident")
        make_identity(nc, ident)
        ones_row = consts.tile([1, 512], BF16, tag="ones_row", name="ones_row")
        nc.vector.memset(ones_row, 1.0)
        ones_sq = consts.tile([P, P], BF16, tag="ones_sq", name="ones_sq")
        nc.vector.memset(ones_sq, 1.0)
        eps_t = consts.tile([P, 1], F32, tag="eps", name="eps")
        nc.vector.memset(eps_t, 1e-5)

        bsh = consts.tile([P, 1], F32, tag="bsh", name="bsh")
        nc.vector.memset(bsh, -2.77258872)  # -4*ln2: ACT exp emits e^s/16

        bqr = consts.tile([1, QK_COLS], BF16, tag="bqr", name="bqr")
        nc.sync.dma_start(out=bqr, in_=bqr_d[:, :])
        bkr = consts.tile([1, QK_COLS], BF16, tag="bkr", name="bkr")
        nc.sync.dma_start(out=bkr, in_=bkr_d[:, :])
        bvt = consts.tile([1, QK_COLS], BF16, tag="bvt", name="bvt")
        nc.sync.dma_start(out=bvt, in_=bv_d[:, :])

        def load_weights():
            # weights arrive bf16 (prescaled by 2^6 / 2^5 host-side so fp8e4
            # normals cover them), cast on-chip (ACT) to fp8 for
            # DoubleRow matmuls.  Called after the first x-tile DMAs are
            # queued so LayerNorm isn't stuck behind 3MB of weights.
            dmas, casts = {}, {}
            for c in range(EC // 2):
                for srcd, nm in ((wv_d, "wv"), (wq_d, "wq"), (wk_d, "wk")):
                    t = persist.tile([P, 2, QK_COLS], BF16, tag=f"{nm}{c}", name=f"{nm}{c}")
                    nc.sync.dma_start(out=t, in_=srcd[c * P:(c + 1) * P, :].rearrange(
                        "p (r m) -> p r m", r=2))
                    t8 = persist.tile([P, 2, QK_COLS], FP8, tag=f"{nm}8{c}", name=f"{nm}8{c}")
                    dmas.setdefault(nm, []).append((t8, t))
            for i in range(2):
                t = persist.tile([P, 2, EMB], BF16, tag=f"wp{i}", name=f"wp{i}")
                nc.sync.dma_start(out=t, in_=wp_d[i * P:(i + 1) * P, :].rearrange(
                    "p (r m) -> p r m", r=2))
                t8 = persist.tile([P, 2, EMB], FP8, tag=f"wp8{i}", name=f"wp8{i}")
                dmas.setdefault("wp", []).append((t8, t))

            def cast_w(nm):
                # fp8 casts deferred so LayerNorm's ACT ops aren't stuck
                # behind 25us of weight casts at kernel start
                out = []
                for t8, t in dmas[nm]:
                    nc.scalar.copy(out=t8, in_=t)
                    out.append(t8)
                return out
            return dmas, cast_w

        # qt: one zero-padded tile per head -- the other head's 64 rows stay
        # zero so ST matmuls can run with the full K=128 kt stationary and
        # keep a single (128,128,512) matmul shape throughout attention
        # (alternating stationary shapes serializes LDWEIGHTS, ~1.5x cost).
        qt = [persist.tile([P, N_TOK], BF16, tag=f"qt{i}", name=f"qt{i}") for i in range(HPC)]
        for i in range(HPC):
            nc.vector.memset(qt[i], 0.0)
        kt = [persist.tile([P, N_TOK], BF16, tag=f"kt{i}", name=f"kt{i}") for i in range(NPAIR)]
        otn = [persist.tile([P, 2, N_TOK], FP8, tag=f"otn{i}", name=f"otn{i}") for i in range(2)]
        # vaug M padded 65 -> 128 with zeros (col 64 = ones for softmax sums)
        vaug = [persist.tile([P, 2, HPC, P], FP8, tag=f"vaug{i}", name=f"vaug{i}")
                for i in range(NT // 2)]
        for t in range(NT // 2):
            nc.vector.memset(vaug[t][:, :, :, HD:], 0.0)
            nc.vector.memset(vaug[t][:, :, :, HD:HD + 1], 1.0)

        # ---------- Phase 1+2: LayerNorm + transpose + V + QK ----------
        ht_ctx = ExitStack()
        ht_pool = ht_ctx.enter_context(tc.tile_pool(name="ht", bufs=1))
        ht = [ht_pool.tile([P, 2, N_TOK], FP8, tag=f"ht{e}", name=f"ht{e}") for e in range(EC // 2)]

        with tc.tile_pool(name="ln", bufs=5) as ln_pool, \
             tc.tile_pool(name="hp", bufs=2) as hp, \
             tc.tile_pool(name="lns", bufs=4) as lns, \
             tc.tile_pool(name="ps_tr", bufs=2, space="PSUM") as ps_tr, \
             tc.tile_pool(name="ps_qkv", bufs=3, space="PSUM") as ps_qkv:
            x_pre = []
            for t in range(4):
                x_t = ln_pool.tile([P, EMB], F32, tag="x", name="x")
                nc.sync.dma_start(out=x_t, in_=x_d[t * P:(t + 1) * P, :])
                x_pre.append(x_t)
            _, cast_w = load_weights()
            wv_s = cast_w("wv")
            wq_s = wk_s = wp_s = None
            for t in range(NT):
                if t == 1:
                    wq_s = cast_w("wq")
                    wk_s = cast_w("wk")
                if t == 8:
                    wp_s = cast_w("wp")
                if t < 4:
                    x_t = x_pre[t]
                else:
                    x_t = ln_pool.tile([P, EMB], F32, tag="x", name="x")
                    nc.sync.dma_start(out=x_t, in_=x_d[t * P:(t + 1) * P, :])
                stats = lns.tile([P, 2, 6], F32, tag="stats", name="stats")
                nc.vector.bn_stats(out=stats[:, 0, :], in_=x_t[:, 0:512])
                nc.vector.bn_stats(out=stats[:, 1, :], in_=x_t[:, 512:1024])
                mv = lns.tile([P, 2], F32, tag="mv", name="mv")
                nc.vector.bn_aggr(out=mv, in_=stats)
                sd = lns.tile([P, 1], F32, tag="sd", name="sd")
                nc.scalar.activation(out=sd, in_=mv[:, 1:2], func=AF.Sqrt,
                                     bias=eps_t, scale=1.0)
                rstd = lns.tile([P, 1], F32, tag="rstd", name="rstd")
                nc.vector.reciprocal(out=rstd, in_=sd)
                nmrs = lns.tile([P, 1], F32, tag="nmrs", name="nmrs")
                nc.vector.scalar_tensor_tensor(
                    out=nmrs, in0=mv[:, 0:1], scalar=-1.0, in1=rstd,
                    op0=ALU.mult, op1=ALU.mult)
                h_bf = hp.tile([P, EMB], BF16, tag="h", name="h")
                nc.scalar.activation(out=h_bf, in_=x_t, func=AF.Identity,
                                     bias=nmrs, scale=rstd)
                # transpose: 4 chunks per PSUM tile, copy out 2x2 chunks
                for half in range(2):
                    pt = ps_tr.tile([P, 4, P], BF16, tag="tr", name="tr")
                    for j in range(4):
                        e = 4 * half + j
                        nc.tensor.transpose(pt[:, j, :],
                                            h_bf[:, e * P:(e + 1) * P], ident)
                    dst0 = ht[2 * half][:, :, t * P:(t + 1) * P]
                    dst1 = ht[2 * half + 1][:, :, t * P:(t + 1) * P]
                    if half == 0:
                        nc.scalar.copy(out=dst0, in_=pt[:, 0:2, :])
                        nc.scalar.copy(out=dst1, in_=pt[:, 2:4, :])
                    else:
                        nc.vector.tensor_copy(out=dst0, in_=pt[:, 0:2, :])
                        nc.vector.tensor_copy(out=dst1, in_=pt[:, 2:4, :])
                # V for this token tile (fp8 DoubleRow over emb pairs)
                pv = ps_qkv.tile([P, 512], F32, tag="qkv", name="qkv")
                for c in range(EC // 2):
                    nc.tensor.matmul(pv, lhsT=ht[c][:, :, t * P:(t + 1) * P],
                                     rhs=wv_s[c], start=(c == 0), stop=False,
                                     perf_mode=DRM)
                nc.tensor.matmul(pv, lhsT=ones_row[:, 0:P], rhs=bvt,
                                 start=False, stop=True)
                nc.vector.tensor_scalar_mul(
                    vaug[t // 2][:, t % 2, :, 0:HD],
                    pv.rearrange("p (h d) -> p h d", h=HPC), 0.03125)
                # QK for the completed 512-token chunk
                if t % 4 == 3:
                    n = t // 4
                    for m in range(NPAIR):
                        for w_s, brow, is_q in ((wq_s, bqr, True),
                                                (wk_s, bkr, False)):
                            pq = ps_qkv.tile([P, 512], F32, tag="qkv", name="qkv")
                            for c in range(EC // 2):
                                nc.tensor.matmul(
                                    pq, lhsT=w_s[c][:, :, m * P:(m + 1) * P],
                                    rhs=ht[c][:, :, n * 512:(n + 1) * 512],
                                    start=(c == 0), stop=False, perf_mode=DRM)
                            nc.tensor.matmul(pq, lhsT=brow[:, m * P:(m + 1) * P],
                                             rhs=ones_row, start=False, stop=True)
                            sl = slice(n * 512, (n + 1) * 512)
                            # undo the 2^6 fp8 weight prescale during copy-out
                            if is_q:
                                nc.scalar.mul(qt[2 * m][0:HD, sl],
                                              pq[0:HD, :], 0.015625)
                                nc.scalar.mul(qt[2 * m + 1][HD:P, sl],
                                              pq[HD:P, :], 0.015625)
                            else:
                                nc.scalar.mul(kt[m][:, sl], pq, 0.015625)

        ht_ctx.close()

        # ---------------- Phase 3: attention (+ proj interleaved) -------
        with tc.tile_pool(name="expp", bufs=2) as expp, \
             tc.tile_pool(name="att_sm", bufs=3) as att_sm, \
             tc.tile_pool(name="zst", bufs=3) as zst, \
             tc.tile_pool(name="ps_st", bufs=4, space="PSUM") as ps_st, \
             tc.tile_pool(name="ps_ot", bufs=2, space="PSUM") as ps_ot, \
             tc.tile_pool(name="ps_misc", bufs=2, space="PSUM") as ps_misc:

            def emit_proj_half(q, ti, half):
                """One projection PSUM group: tokens [128], emb cols [512]."""
                tt = q * 4 + ti
                pz = ps_misc.tile([P, 512], F32, tag="misc", name="pz")
                for cc in range(2):
                    nc.tensor.matmul(
                        pz, lhsT=otn[cc][:, :, tt * P:(tt + 1) * P],
                        rhs=wp_s[cc][:, :, half * 512:(half + 1) * 512],
                        start=(cc == 0), stop=(cc == 1), perf_mode=DRM)
                z_t = zst.tile([P, 512], F32, tag="z", name="z")
                # undo the 2^5 fp8 w_proj prescale
                nc.vector.tensor_scalar_mul(z_t, pz, 0.03125)
                nc.sync.dma_start(
                    out=z_d[tt * P:(tt + 1) * P, half * 512:(half + 1) * 512],
                    in_=z_t)

            # rec_pad slots: rows 1..127 must stay zero so the ones_sq
            # broadcast matmul (K=128, same shape as ST/AV) sees only row 0.
            rec_pads = []
            for j in range(3):
                rp = att_sm.tile([P, 512], BF16, tag=f"rec_pad{j}",
                                 name=f"rec_pad{j}", bufs=1)
                nc.vector.memset(rp, 0.0)
                rec_pads.append(rp)
            rp_idx = [0]

            def stage_a(u):
                """After AV stop: 1/Z straight off the PSUM sums row, cast on
                gpsimd, raw attention rows to SBUF on ACT."""
                q, h, pot = u
                srow = att_sm.tile([1, 512], F32, tag="srow", name="srow")
                nc.scalar.copy(out=srow, in_=pot[HD:HD + 1, :])
                rec = att_sm.tile([1, 512], F32, tag="rec", name="rec")
                nc.vector.reciprocal_approx_fast(out=rec, in_=srow)
                rec_pad = rec_pads[rp_idx[0] % 3]
                rp_idx[0] += 1
                nc.scalar.copy(out=rec_pad[0:1, :], in_=rec)
                ot_raw = att_sm.tile([HD, 512], BF16, tag="ot_raw", name="ot_raw")
                nc.scalar.copy(out=ot_raw, in_=pot[0:HD, :])
                return (q, h, rec_pad, ot_raw)

            def stage_b(u):
                """Two units later: broadcast 1/Z on PE, multiply into otn."""
                q, h, rec_pad, ot_raw = u
                pb = ps_misc.tile([P, 512], F32, tag="misc", name="pb")
                nc.tensor.matmul(pb, lhsT=ones_sq, rhs=rec_pad,
                                 start=True, stop=True)
                nc.vector.tensor_mul(
                    otn[h // 4][(h % 2) * HD:(h % 2) * HD + HD, (h // 2) % 2,
                                q * 512:(q + 1) * 512],
                    ot_raw, pb[0:HD, :])

            units = [(q, h) for q in range(QCH) for h in range(HPC)]
            av_prev = None      # (q, h, e_tiles) awaiting AV during this unit
            sa_queue = []       # stage-A results awaiting stage B (2-deep)
            proj_queue = []     # (q, ti, half) proj groups ready to emit

            def do_stage_b(sb):
                stage_b(sb)
                if sb[1] == HPC - 1:
                    proj_queue.extend(
                        (sb[0], ti, half) for ti in range(4) for half in range(2))

            for q, h in units:
                pair = h // 2
                # start-of-unit: one deferred normalize + one proj group.
                # both have inputs computed >= one full unit ago, so the PE
                # instructions here never wait on ACT/DVE.
                if len(sa_queue) >= 2:
                    do_stage_b(sa_queue.pop(0))
                if proj_queue:
                    emit_proj_half(*proj_queue.pop(0))
                # ACT k-tiles 0..7 as fp8 pairs (AV consumes via DoubleRow),
                # DVE k-tiles 8..15 as bf16 singles (AV consumes via bf16 mm)
                e8p = [expp.tile([P, 2, 512], FP8, tag=f"e8p{j}", name=f"e8p{j}")
                       for j in range(NKT // 2)]
                pot_prev = None
                if av_prev is not None:
                    pot_prev = ps_ot.tile([P, 512], F32, tag="ot", name="ot")

                def emit_av(c):
                    if c % 2:
                        return  # consumed by the pair's DR matmul
                    ph_, pe8_ = av_prev[1], av_prev[2]
                    nc.tensor.matmul(
                        pot_prev, lhsT=vaug[c // 2][:, :, ph_, :],
                        rhs=pe8_[c // 2], start=(c == 0),
                        stop=(c == NKT - 2), perf_mode=DRM)

                for c in range(NKT):
                    pst = ps_st.tile([P, 512], F32, tag="st", name="st")
                    nc.tensor.matmul(
                        pst,
                        lhsT=kt[pair][:, c * P:(c + 1) * P],
                        rhs=qt[h][:, q * 512:(q + 1) * 512],
                        start=True, stop=True)
                    if av_prev is not None:
                        emit_av(c)
                    if c < N_ACT_EXP:
                        nc.scalar.activation(out=e8p[c // 2][:, c % 2, :],
                                             in_=pst, func=AF.Exp, bias=bsh,
                                             scale=1.0 / A_EXP)
                    else:
                        nc.vector.tensor_scalar(
                            out=e8p[c // 2][:, c % 2, :].bitcast(I8), in0=pst,
                            scalar1=B_SCHR8, scalar2=0.0,
                            op0=ALU.add, op1=ALU.max)
                # end-of-unit: AV of av_prev just completed -> stage A now
                if av_prev is not None:
                    sa_queue.append(stage_a((av_prev[0], av_prev[1], pot_prev)))
                av_prev = (q, h, e8p)

            # drain: AV for the last unit, then remaining finalize stages
            if len(sa_queue) >= 2:
                do_stage_b(sa_queue.pop(0))
            pot_last = ps_ot.tile([P, 512], F32, tag="ot", name="ot")
            ph_, pe8_ = av_prev[1], av_prev[2]
            for j in range(NKT // 2):
                nc.tensor.matmul(pot_last, lhsT=vaug[j][:, :, ph_, :],
                                 rhs=pe8_[j], start=(j == 0),
                                 stop=(j == NKT // 2 - 1), perf_mode=DRM)
            sa_queue.append(stage_a((av_prev[0], av_prev[1], pot_last)))
            while sa_queue:
                do_stage_b(sa_queue.pop(0))
                if proj_queue:
                    emit_proj_half(*proj_queue.pop(0))
            for g in proj_queue:
                emit_proj_half(*g)
            proj_queue.clear()


_CACHE = {}


def _get_nc():
    if "nc" not in _CACHE:
        _CACHE["nc"] = build_nc()
    return _CACHE["nc"]


def _prep_in_maps(x, ln_w, ln_b, w_qkv, b_qkv, w_proj, b_proj):
    bf = ml_dtypes.bfloat16
    x = np.asarray(x, np.float32)
    ln_w = np.asarray(ln_w, np.float32)
    ln_b = np.asarray(ln_b, np.float32)
    w_qkv = np.asarray(w_qkv, np.float32)
    b_qkv = np.asarray(b_qkv, np.float32)
    w_proj = np.asarray(w_proj, np.float32)

    b_eff = b_qkv + ln_b @ w_qkv
    w_eff = ln_w[:, None] * w_qkv
    w4 = w_eff.reshape(EMB, HEADS, HD, 3)
    b4 = b_eff.reshape(HEADS, HD, 3)
    # sqrt(softmax scale) folded into both q and k; 2^6 (qk) / 2^5 (v, proj)
    # prescales lift the weights into fp8e4's normal range -- the kernel
    # multiplies the PSUM results by the inverse power of two on copy-out.
    sq_s = SCALE ** 0.5
    wq = w4[..., 0] * (sq_s * 64.0 * A_EXP)
    wk = w4[..., 1] * (sq_s * 64.0)
    wv = w4[..., 2] * 32.0
    bq = b4[..., 0] * (sq_s * 64.0 * A_EXP)
    bk = b4[..., 1] * (sq_s * 64.0)
    bv = b4[..., 2] * 32.0
    w_proj = w_proj * 32.0

    def _dr(w):
        # [R, M] -> [R/2, 2M]: row 256c+128r+k -> (c*128+k, r*M+m)
        R, M = w.shape
        return np.ascontiguousarray(
            w.reshape(R // 256, 2, 128, M).transpose(0, 2, 1, 3).reshape(R // 2, 2 * M))

    in_maps = []
    for cid in range(N_CORES):
        bi, hg = divmod(cid, 2)
        hsl = slice(hg * HPC, (hg + 1) * HPC)
        in_maps.append({
            "x": np.ascontiguousarray(x[bi]),
            "wq": _dr(wq[:, hsl, :].reshape(EMB, QK_COLS)).astype(bf),
            "wk": _dr(wk[:, hsl, :].reshape(EMB, QK_COLS)).astype(bf),
            "wv": _dr(wv[:, hsl, :].reshape(EMB, QK_COLS)).astype(bf),
            "bqr": np.ascontiguousarray(
                bq[hsl].reshape(1, QK_COLS)).astype(bf),
            "bkr": np.ascontiguousarray(
                bk[hsl].reshape(1, QK_COLS)).astype(bf),
            "bv": np.ascontiguousarray(
                bv[hsl].reshape(1, QK_COLS)).astype(bf),
            "wp": _dr(w_proj[hg * QK_COLS:(hg + 1) * QK_COLS, :]).astype(bf),
        })
    return in_maps


def _gather(results, x, b_proj):
    b_proj = np.asarray(b_proj, np.float32)
    x = np.asarray(x, np.float32)
    out = np.empty((x.shape[0], N_TOK, EMB), np.float32)
    for bi in range(x.shape[0]):
        out[bi] = (results[2 * bi]["z"] + results[2 * bi + 1]["z"]
                   + b_proj[None, :] + x[bi])
    return out


def _run(inputs, **kw):
    in_maps = _prep_in_maps(**inputs)
    res = run_bass_kernel_spmd(_get_nc(), in_maps,
                               core_ids=list(range(N_CORES)), **kw)
    out = _gather(res.results, inputs["x"], inputs["b_proj"])
    return out, res


def kernel(**inputs):
    out, _ = _run(inputs)
    return out
